# revision 50
# baseline (speedup 1.0000x reference)
"""Memory-efficient multi-head cross-attention on 8 TRN2 NeuronCores.

Sharding: each core owns 2 heads ({2c, 2c+1}) for BOTH batches
(tensor-parallel qkv projections over the head axis).  Per 512-row q chunk,
each core normalizes its context and a single dense 8-core AllToAll exchanges
[head-block x (batch, q-block)] tiles, after which every core holds the full
1024-channel context for its own (batch c//4, q rows 512*jc + 128*(c%4)).
The o-projection, residual add and LayerNorm then run fully locally -- no
cross-core reduction of o-proj partials is needed.

Pipeline (per chunk jc): attention(jc) emits Q'proj(jc+1), LN-stats(jc-2) and
o-proj(jc-1) as PE/DVE filler inside the (ACT-exp-bound) attention inner loop;
the A2A for chunk jc triggers as soon as both batches' context is normalized.
LayerNorm's Sqrt-dependent finish runs at the tail to avoid ACT table-set
switches between Exp and Sqrt.

kernel(**inputs) takes the FULL unsharded inputs and returns the FULL output.
"""

import sys
import types

import ml_dtypes
import numpy as np

# ---------------------------------------------------------------------------
# Environment shims (must run before concourse imports are used)
# ---------------------------------------------------------------------------


def _install_ntff_shim():
    """Provide antenv.axon_hooks (absent in this image) so that
    run_bass_kernel_spmd(trace=True) can capture NTFF profiles via the
    axon ctypes hook. Harmless when tracing is off."""
    if "antenv.axon_hooks" in sys.modules:
        return
    hook = None
    try:
        from trn_agent_boot.trn_boot import _ntff_profile_via_ctypes

        hook = _ntff_profile_via_ctypes("/opt/axon/libaxon_pjrt.so")
    except Exception:
        hook = None
    mod = types.ModuleType("antenv.axon_hooks")
    mod.get_axon_ntff_profile_hook = lambda: hook
    mod.set_axon_ntff_profile_hook = lambda h: None
    sys.modules["antenv.axon_hooks"] = mod


_install_ntff_shim()

import concourse.bass as bass  # noqa: E402
import concourse.mybir as mybir  # noqa: E402
import concourse.tile as tile  # noqa: E402
from concourse.bass_utils import run_bass_kernel_spmd  # noqa: E402
from concourse.vector_clock import ScopedClock  # noqa: E402
import bass_rust as _bass_rust  # noqa: E402


def _patched_drain_and_barrier(self, tick_clock, wait_clock):
    """The walrus build in this image rejects a Drain carrying multiple sem
    waits ("Too many sync wait commands").  Emit the kernel-tail waits as
    standalone wait instructions on the sync engine instead, then drain."""
    nc = self.nc
    probe = nc.sync.nop(nofuse=True)
    wait_clock.add_sem_waits(probe.ins, ScopedClock({None: tick_clock.global_clock}))
    waits = list(probe.ins.sync_info.on_wait)
    probe.ins.sync_info.on_wait.clear()
    name2sem = {s.name: s for s in self.sems.allocated().values()}
    for w in waits:
        nc.sync.wait_ge(name2sem[w.ant_name], w.wait_value)
    nc.sync.drain()
    nc.all_engine_barrier()
    popped = nc._tile_sem_poison_stack.pop()
    assert popped is self._sem_poison
    nc.clear_and_free_semaphores(list(self.sems.allocated().values()))
    nc.all_engine_barrier()


tile.TileContext._drain_and_barrier = _patched_drain_and_barrier

# Max sem-waits this walrus build accepts on a single instruction.
_WAIT_LIMIT = 1


def _split_waits(nc, limit=_WAIT_LIMIT):
    """Hoist excess per-instruction sem waits into standalone EventSemaphore
    instructions (same engine, immediately preceding), since this walrus build
    rejects instructions carrying more than one sync wait."""
    n_split = 0
    for f in nc.m.functions:
        for bb in f.blocks:
            insts = bb.instructions
            i = 0
            while i < len(insts):
                inst = insts[i]
                si = getattr(inst, "sync_info", None)
                waits = si.on_wait if si is not None else None
                if waits is not None and len(waits) > limit:
                    excess = list(waits)[limit:]
                    del waits[limit:]
                    for w in excess:
                        ev = mybir.InstEventSemaphore(
                            name=f"I-{nc.next_id()}",
                            engine=inst.engine,
                            ins=[],
                            outs=[],
                        )
                        ev.sync_info = mybir.SyncInfo(on_wait=[w], on_update=[])
                        insts.insert(i, ev)
                        i += 1
                        n_split += 1
                i += 1
    return n_split


# ---------------------------------------------------------------------------
# Problem constants (hardcoded per the harness contract)
# ---------------------------------------------------------------------------
B = 2
SQ = 2048
SKV = 2048
D = 1024
NH = 16
DK = 64

NCORES = 8
GSZ = 4  # q-blocks per chunk / cores per batch in the output mapping
DLOC = 2 * DK  # 128 local context channels (2 heads)
P = 128
QCH = 512  # q chunk (matmul moving free dim)
NQC = SQ // QCH  # 4
NKT = SKV // P  # 16 k tiles
NMT = D // P  # 8 contraction tiles over model dim

F32 = mybir.dt.float32
BF16 = mybir.dt.bfloat16

LN_EPS = 1e-5

_CACHE = {}
LAST_RESULT = None


def _build():
    """Build the SPMD Bass program (identical on all 8 cores)."""
    nc = bass.Bass("TRN2", target_bir_lowering=False, num_devices=NCORES)

    # ---- kernel I/O (per-core shards supplied by the host) ----
    # x inputs are full (both batches), transposed: [b, d, seq]
    xqT = nc.dram_tensor("xqT", [B, D, SQ], BF16, kind="ExternalInput")
    xkvT = nc.dram_tensor("xkvT", [B, D, SKV], BF16, kind="ExternalInput")
    # projection weights for this core's 2 heads: [d_in, 128]
    wqT = nc.dram_tensor("wqT", [D, DLOC], BF16, kind="ExternalInput")
    wkT = nc.dram_tensor("wkT", [D, DLOC], BF16, kind="ExternalInput")
    wvT = nc.dram_tensor("wvT", [D, DLOC], BF16, kind="ExternalInput")
    bqs = nc.dram_tensor("bqs", [P, 1], F32, kind="ExternalInput")
    bks = nc.dram_tensor("bks", [P, 1], F32, kind="ExternalInput")
    bvr = nc.dram_tensor("bvr", [1, DLOC], BF16, kind="ExternalInput")
    # full o-proj weights W_o.T [d_in, n]; rows 128*s.. = heads {2s, 2s+1}
    woT = nc.dram_tensor("woT", [D, D], BF16, kind="ExternalInput")
    # residual rows (query + b_o) for this core: [jc, 128, D]
    qres = nc.dram_tensor("qres", [NQC, P, D], F32, kind="ExternalInput")
    gam = nc.dram_tensor("gam", [P, D], F32, kind="ExternalInput")
    bet = nc.dram_tensor("bet", [P, D], F32, kind="ExternalInput")
    out = nc.dram_tensor("out", [NQC, P, D], F32, kind="ExternalOutput")

    groups = [[0, 1, 2, 3, 4, 5, 6, 7]]
    Exp = mybir.ActivationFunctionType.Exp
    Sqrt = mybir.ActivationFunctionType.Sqrt
    MULT = mybir.AluOpType.mult
    ADD = mybir.AluOpType.add

    xq_v = xqT.ap().rearrange("b (t p) q -> p b t q", p=P)
    xkv_v = xkvT.ap().rearrange("b (t p) q -> p b t q", p=P)

    with tile.TileContext(nc) as tc:
        with (
            tc.tile_pool(name="cpool", bufs=1) as cpool,
            tc.tile_pool(name="spool", bufs=2) as spool,
            tc.tile_pool(name="dram", bufs=1, space="DRAM") as dram,
        ):
            # ---- persistent SBUF tensors ----
            wq_sb = cpool.tile([P, NMT, DLOC], BF16)
            wk_sb = cpool.tile([P, NMT, DLOC], BF16)
            wv_sb = cpool.tile([P, NMT, DLOC], BF16)
            bqs_sb = cpool.tile([P, 1], F32)
            bks_sb = cpool.tile([P, 1], F32)
            bvr_sb = cpool.tile([1, DLOC], BF16)
            onesP = cpool.tile([P, P], BF16)
            eps_sb = cpool.tile([P, 1], F32)
            # K'^T per batch (d-of-head-pair on partitions)
            kt_sb = cpool.tile([P, B, SKV], BF16)
            v_sb = cpool.tile([P, B, NKT, DLOC], BF16)  # V rows (k on partitions)
            wo_sb = cpool.tile([P, NMT, D], BF16)  # W_o^T, slab s = heads {2s,2s+1}
            gam_sb = cpool.tile([P, D], F32)
            bet_sb = cpool.tile([P, D], F32)

            # Queue budget: sync+scalar carry the 8MB xkv stream (split); the
            # gpsimd queue issues the secondary loads as doorbells before the
            # warmup collective's completion wait blocks it.
            nc.sync.dma_start(wk_sb[:], wkT.ap().rearrange("(t p) d -> p t d", p=P))
            nc.sync.dma_start(bks_sb[:], bks.ap())
            nc.scalar.dma_start(wv_sb[:], wvT.ap().rearrange("(t p) d -> p t d", p=P))
            nc.scalar.dma_start(bvr_sb[:], bvr.ap())
            nc.vector.memset(onesP[:], 1.0)
            nc.vector.memset(eps_sb[:], LN_EPS)

            # xq chunk ring: [P, b, mt, q] for one chunk
            def xq_fetch(jc):
                xq_r = spool.tile(
                    [P, B, NMT, QCH], BF16, tag="xqr", name=f"xqr_{jc}", bufs=1
                )
                nc.sync.dma_start(
                    xq_r[:], xq_v[:, :, :, QCH * jc : QCH * jc + QCH]
                )
                return xq_r

            # residual rows ring: [P, D] for one chunk
            def qres_fetch(jc):
                qres_r = spool.tile(
                    [P, D], F32, tag="qres", name=f"qres_{jc}", bufs=3
                )
                nc.sync.dma_start(qres_r[:], qres.ap()[jc])
                return qres_r

            # ---------------- Phase A: K'/V projections ----------------
            # Warm up the collectives stream early: a tiny AllToAll during
            # phase A absorbs the ~30us first-collective ncfw startup cost
            # that would otherwise stall the chunk-1 o-proj pipeline.
            warm_sb = cpool.tile([P, NCORES, P], BF16)
            nc.vector.memset(warm_sb[:], 0.0)
            warm_in = dram.tile([NCORES * P, P], BF16, name="warm_in")
            warm_out = dram.tile([NCORES * P, P], BF16, name="warm_out")
            nc.gpsimd.dma_start(
                warm_in.rearrange("(s p) q -> p s q", p=P), warm_sb[:]
            )
            nc.gpsimd.collective_compute(
                "AllToAll",
                mybir.AluOpType.bypass,
                replica_groups=[[0, 1, 2, 3, 4, 5, 6, 7]],
                ins=[warm_in.opt()],
                outs=[warm_out.opt()],
            )

            xq_rings = {}
            with (
                tc.tile_pool(name="apool", bufs=1) as apool,
                tc.tile_pool(name="psA", bufs=8, space="PSUM") as psA,
            ):
                # spread the bulk xkv stream over two engines' hardware DMA
                # queues -- a single engine's queue tops out well below HBM
                # bandwidth
                xkv_tiles = []
                last_dma = {}
                for b in range(B):
                    xkv_sb = apool.tile(
                        [P, NMT, SKV], BF16, tag="xkv", name=f"xkv_{b}", bufs=2
                    )
                    xkv_tiles.append(xkv_sb)
                    for mt in range(NMT):
                        eng = "sync" if mt % 2 == 0 else "scalar"
                        last_dma[eng] = getattr(nc, eng).dma_start(
                            xkv_sb[:, mt, :], xkv_v[:, b, mt, :]
                        )
                # secondary loads behind the xkv stream, order pinned so the
                # scheduler cannot hoist their transfers ahead of it
                xq_rings[0] = xq_fetch(0)
                for dma in (
                    nc.scalar.dma_start(
                        wq_sb[:], wqT.ap().rearrange("(t p) d -> p t d", p=P)
                    ),
                    nc.scalar.dma_start(bqs_sb[:], bqs.ap()),
                    nc.scalar.dma_start(
                        wo_sb[:], woT.ap().rearrange("(t p) n -> p t n", p=P)
                    ),
                    nc.scalar.dma_start(gam_sb[:], gam.ap()),
                    nc.scalar.dma_start(bet_sb[:], bet.ap()),
                ):
                    _bass_rust.add_dep_helper(
                        dma.ins, last_dma["scalar"].ins, False, "dma order"
                    )
                for b in range(B):
                    xkv_sb = xkv_tiles[b]

                    # K'^T: out[dloc(128), kv(512)], m-tile outer so matmuls
                    # start as soon as the first input chunk lands.
                    pss = [
                        psA.tile([P, QCH], F32, tag="pj", name=f"pk_{b}_{i}")
                        for i in range(NQC)
                    ]
                    for mt in range(NMT):
                        for qc in range(NQC):
                            nc.tensor.matmul(
                                pss[qc][:],
                                lhsT=wk_sb[:, mt, :],
                                rhs=xkv_sb[:, mt, QCH * qc : QCH * qc + QCH],
                                start=(mt == 0),
                                stop=(mt == NMT - 1),
                            )
                    for qc in range(NQC):
                        nc.vector.tensor_scalar(
                            kt_sb[:, b, QCH * qc : QCH * qc + QCH],
                            pss[qc][:],
                            1.0,
                            bks_sb[:, 0:1],
                            MULT,
                            ADD,
                        )

                    # V: out[k_tile(128), dloc(128)] over m-tiles + bias row.
                    for kt in range(NKT):
                        ps = psA.tile([P, QCH], F32, tag="pj", name=f"pv_{b}_{kt}")
                        pv = ps[:, :DLOC]
                        for mt in range(NMT):
                            nc.tensor.matmul(
                                pv,
                                lhsT=xkv_sb[:, mt, P * kt : P * kt + P],
                                rhs=wv_sb[:, mt, :],
                                start=(mt == 0),
                                stop=False,
                            )
                        nc.tensor.matmul(
                            pv,
                            lhsT=onesP[0:1, :],
                            rhs=bvr_sb[0:1, :],
                            start=False,
                            stop=True,
                        )
                        nc.vector.tensor_copy(v_sb[:, b, kt, :], pv)

            # ------- Phase B: per-chunk attention + A2A + o-proj + LN -------
            with (
                tc.tile_pool(name="bpool", bufs=1) as bpool,
                tc.tile_pool(name="psB", bufs=1, space="PSUM") as psB,
            ):
                # exp(scores): [k(128), ktp, head*kt-in-pair, q]
                p_sb = bpool.tile([P, NKT // 2, 4, QCH], BF16)

                # The tile scheduler reorders instructions; filler steps must
                # be pinned behind the attention matmuls they are meant to
                # pad, or they get hoisted to the chunk start where their
                # input waits (collectives, DMAs) block the in-order queues.
                anchor = {}

                def pin(inst):
                    if "pe" in anchor:
                        _bass_rust.add_dep_helper(
                            inst.ins, anchor["pe"].ins, False, "filler order"
                        )
                    return inst

                cc_out = {}
                qt_rings = {}
                qres_rings = {}
                x_tiles = {}
                ln_st = {}

                def make_qproj_steps(jc):
                    """Q'^T projection for chunk jc: 4 filler steps."""
                    st = {}
                    qt_rings[jc] = spool.tile(
                        [P, B, QCH], BF16, tag="qt", name=f"qt_{jc}", bufs=2
                    )

                    def mm_step(b, half):
                        if half == 0:
                            st[b] = psB.tile(
                                [P, QCH], F32, tag="fill", name=f"pq_{jc}_{b}", bufs=1
                            )
                        pq = st[b]
                        xq_r = xq_rings[jc]
                        for mt in range(4 * half, 4 * half + 4):
                            pin(
                                nc.tensor.matmul(
                                    pq[:],
                                    lhsT=wq_sb[:, mt, :],
                                    rhs=xq_r[:, b, mt, :],
                                    start=(mt == 0),
                                    stop=(mt == NMT - 1),
                                )
                            )
                        if half == 1:
                            nc.vector.tensor_scalar(
                                qt_rings[jc][:, b, :],
                                pq[:],
                                0.125,
                                bqs_sb[:, 0:1],
                                MULT,
                                ADD,
                            )

                    return [
                        (lambda b=b, h=h: mm_step(b, h))
                        for b in range(B)
                        for h in range(2)
                    ]

                def make_oproj_steps(jc):
                    """A2A receive + o-projection for chunk jc: 5 steps."""
                    st = {}

                    def dmain_step():
                        # on the scalar queue: its HWDGE doorbell is ~10x
                        # cheaper than gpsimd's SWDGE descriptor build, and
                        # the cc_out it waits on is a 2-chunk-old collective,
                        # so it never stalls the exp stream behind it.
                        ctg = spool.tile(
                            [P, NCORES, P], BF16, tag="ctg", name=f"ctg_{jc}", bufs=2
                        )
                        st["ctg"] = ctg
                        pin(
                            nc.scalar.dma_start(
                                ctg[:], cc_out[jc].rearrange("(s p) q -> p s q", p=P)
                            )
                        )

                    def mm_step(nch, half):
                        if nch == 0 and half == 0:
                            x_tiles[jc] = spool.tile(
                                [P, D], F32, tag="x", name=f"x_{jc}", bufs=2
                            )
                        if half == 0:
                            st["po"] = psB.tile(
                                [P, QCH], F32, tag="fill", name=f"po_{jc}_{nch}", bufs=1
                            )
                        po = st["po"]
                        ctg = st["ctg"]
                        nsl = slice(QCH * nch, QCH * nch + QCH)
                        for s in range(4 * half, 4 * half + 4):
                            pin(
                                nc.tensor.matmul(
                                    po[:],
                                    lhsT=ctg[:, s],
                                    rhs=wo_sb[:, s, nsl],
                                    start=(s == 0),
                                    stop=(s == NCORES - 1),
                                )
                            )
                        if half == 1:
                            nc.vector.tensor_add(
                                x_tiles[jc][:, nsl], po[:], qres_rings[jc][:, nsl]
                            )

                    return [dmain_step] + [
                        (lambda n=n, h=h: mm_step(n, h))
                        for n in range(2)
                        for h in range(2)
                    ]

                def make_ln_stat_steps(jc):
                    """LayerNorm statistics for chunk jc (DVE only): 2 steps."""
                    st = {}
                    ln_st[jc] = st

                    def s1():
                        x = x_tiles[jc]
                        mean = spool.tile(
                            [P, 1], F32, tag="mean", name=f"mean_{jc}", bufs=4
                        )
                        nmean = spool.tile(
                            [P, 1], F32, tag="nmean", name=f"nmean_{jc}", bufs=4
                        )
                        xc = spool.tile([P, D], F32, tag="xc", name=f"xc_{jc}", bufs=3)
                        st["xc"] = xc
                        pin(
                            nc.vector.reduce_sum(
                                mean[:], x[:], axis=mybir.AxisListType.X
                            )
                        )
                        nc.vector.tensor_scalar_mul(nmean[:], mean[:], -1.0 / D)
                        nc.vector.tensor_scalar_add(xc[:], x[:], nmean[:])

                    def s2():
                        xc = st["xc"]
                        sq = spool.tile([P, D], F32, tag="sq", name=f"sq_{jc}", bufs=1)
                        ssq = spool.tile(
                            [P, 1], F32, tag="ssq", name=f"ssq_{jc}", bufs=4
                        )
                        st["ssq"] = ssq
                        nc.vector.tensor_mul(sq[:], xc[:], xc[:])
                        nc.vector.reduce_sum(ssq[:], sq[:], axis=mybir.AxisListType.X)

                    return [s1, s2]

                def ln_finish(jc):
                    """Sqrt + scale/shift + output DMA for chunk jc (tail)."""
                    xc = ln_st[jc]["xc"]
                    ssq = ln_st[jc]["ssq"]
                    sd = spool.tile([P, 1], F32, tag="sd", name=f"sd_{jc}", bufs=2)
                    rstd = spool.tile(
                        [P, 1], F32, tag="rstd", name=f"rstd_{jc}", bufs=2
                    )
                    y = spool.tile([P, D], F32, tag="y", name=f"y_{jc}", bufs=2)
                    nc.scalar.activation(
                        sd[:], ssq[:], Sqrt, scale=1.0 / D, bias=eps_sb[:]
                    )
                    nc.vector.reciprocal(rstd[:], sd[:])
                    nc.vector.tensor_scalar_mul(y[:], xc[:], rstd[:])
                    nc.vector.tensor_mul(y[:], y[:], gam_sb[:])
                    nc.vector.tensor_add(y[:], y[:], bet_sb[:])
                    nc.sync.dma_start(out.ap()[jc], y[:])

                def attn_chunk(jc, fillers):
                    # prefetch next chunk's x_q slice + this chunk's residual
                    if jc + 1 < NQC:
                        xq_rings[jc + 1] = xq_fetch(jc + 1)
                    qres_rings[jc] = qres_fetch(jc)
                    ct = spool.tile(
                        [P, B, GSZ, P], BF16, tag="ct", name=f"ct_{jc}", bufs=2
                    )
                    cc_in = dram.tile([NCORES * P, P], BF16, name=f"cc_in_{jc}")
                    cc_in_v = cc_in.rearrange("(d p) q -> p d q", p=P)
                    qt_r = qt_rings[jc]
                    for b in range(B):
                        if b == 1:
                            # bridge the b0->b1 softmax-reciprocal WAR chain
                            # with PE filler so HAM stays warm
                            for _ in range(2):
                                if fillers:
                                    fillers.pop(0)()
                        ctx = psB.tile(
                            [P, QCH], F32, tag="ctx", name=f"ctx_{jc}_{b}", bufs=2
                        )
                        dn = psB.tile(
                            [P, QCH], F32, tag="dn", name=f"dn_{jc}_{b}", bufs=1
                        )

                        def scores(kt):
                            # one kt (both heads) per 2-bank psum slab; slabs
                            # ping-pong (tags sA/sB) so the next kt's matmuls
                            # run while ACT still exps the previous slab.
                            j = kt % 2
                            s2 = psB.tile(
                                [P, 2, QCH],
                                F32,
                                tag=("sA", "sB")[j],
                                name=f"s_{jc}_{b}_{kt}",
                                bufs=1,
                            )
                            ksl = slice(P * kt, P * kt + P)
                            nc.tensor.matmul(
                                s2[:, 0, :],
                                lhsT=kt_sb[0:DK, b, ksl],
                                rhs=qt_r[0:DK, b, :],
                            )
                            anchor["pe"] = nc.tensor.matmul(
                                s2[:, 1, :],
                                lhsT=kt_sb[DK:P, b, ksl],
                                rhs=qt_r[DK:P, b, :],
                            )
                            nc.scalar.activation(
                                p_sb[:, kt // 2, 2 * j : 2 * j + 2], s2[:], Exp
                            )

                        def ctx2(kt):
                            ktp, j = kt // 2, kt % 2
                            st_, sp = kt == 0, kt == NKT - 1
                            nc.tensor.matmul(
                                ctx[0:DK, :],
                                lhsT=v_sb[:, b, kt, 0:DK],
                                rhs=p_sb[:, ktp, 2 * j, :],
                                start=st_,
                                stop=sp,
                            )
                            nc.tensor.matmul(
                                ctx[DK:P, :],
                                lhsT=v_sb[:, b, kt, DK:P],
                                rhs=p_sb[:, ktp, 2 * j + 1, :],
                                start=st_,
                                stop=sp,
                            )
                            # denominator rows, broadcast across the head's 64
                            # partitions by the ones stationary (same
                            # 512-cycle stream as a 1-row output)
                            nc.tensor.matmul(
                                dn[0:DK, :],
                                lhsT=onesP[:, 0:DK],
                                rhs=p_sb[:, ktp, 2 * j, :],
                                start=st_,
                                stop=sp,
                            )
                            anchor["pe"] = nc.tensor.matmul(
                                dn[DK:P, :],
                                lhsT=onesP[:, DK:P],
                                rhs=p_sb[:, ktp, 2 * j + 1, :],
                                start=st_,
                                stop=sp,
                            )

                        scores(0)
                        scores(1)
                        for kt2 in range(2, NKT, 2):
                            scores(kt2)
                            scores(kt2 + 1)
                            ctx2(kt2 - 2)
                            ctx2(kt2 - 1)
                            if kt2 >= 4 and fillers:
                                fillers.pop(0)()
                        ctx2(NKT - 2)
                        ctx2(NKT - 1)

                        # normalize: ct = ctx * (1/dn), then ship to DRAM for
                        # the A2A (dn rows are already broadcast per head)
                        rd = spool.tile(
                            [P, QCH], F32, tag="rd", name=f"rd_{jc}_{b}", bufs=2
                        )
                        nc.vector.reciprocal(rd[:], dn[:])
                        nc.vector.tensor_mul(ct[:, b], ctx[:], rd[:])
                        nc.sync.dma_start(
                            cc_in_v[:, GSZ * b : GSZ * b + GSZ, :], ct[:, b]
                        )

                    cc_out[jc] = dram.tile([NCORES * P, P], BF16, name=f"cc_out_{jc}")
                    nc.gpsimd.collective_compute(
                        "AllToAll",
                        mybir.AluOpType.bypass,
                        replica_groups=groups,
                        ins=[cc_in.opt()],
                        outs=[cc_out[jc].opt()],
                    )

                    while fillers:
                        fillers.pop(0)()

                # Q'(0) before the first chunk
                for step in make_qproj_steps(0):
                    step()

                def make_warm_steps(jc, n):
                    # spare PE work for thin early chunks: keeps HAM at full
                    # clock across the softmax-reciprocal boundary stalls
                    def wstep(i):
                        wm = psB.tile(
                            [P, QCH], F32, tag="fill", name=f"wf_{jc}_{i}", bufs=1
                        )
                        for _ in range(2):
                            pin(
                                nc.tensor.matmul(
                                    wm[:], lhsT=onesP[:], rhs=wo_sb[:, 0, 0:QCH]
                                )
                            )

                    return [(lambda i=i: wstep(i)) for i in range(n)]

                for jc in range(NQC):
                    # o-proj for chunk jc-2: its A2A has had a full chunk to
                    # complete, so these steps never stall the PE queue even
                    # when a collective runs long.
                    fillers = []
                    if jc >= 2:
                        fillers += make_oproj_steps(jc - 2)
                    if jc < NQC - 1:
                        fillers += make_qproj_steps(jc + 1)
                    if jc >= 2:
                        fillers += make_ln_stat_steps(jc - 2)
                    else:
                        fillers += make_warm_steps(jc, 5)
                    attn_chunk(jc, fillers)

                # ---- tail ----
                # chunk 2's o-proj (A2A long done) + LN finishes overlap the
                # last chunk's A2A; keep-warm matmuls hold the PE at full
                # clock across the collective wait so o-proj(3) runs warm.
                ln_finish(0)
                ln_finish(1)
                for step in make_oproj_steps(2):
                    step()
                for step in make_ln_stat_steps(2):
                    step()
                ln_finish(2)
                for i in range(28):
                    wm = psB.tile(
                        [P, QCH], F32, tag="fill", name=f"warmmm_{i}", bufs=1
                    )
                    pin(
                        nc.tensor.matmul(
                            wm[:], lhsT=onesP[:], rhs=wo_sb[:, 0, 0:QCH]
                        )
                    )
                for step in make_oproj_steps(NQC - 1):
                    step()
                for step in make_ln_stat_steps(NQC - 1):
                    step()
                ln_finish(NQC - 1)

    _split_waits(nc)
    return nc


def _prep_inputs(query, key_value, W_qkv, b_qkv, W_o, b_o, ln_gamma, ln_beta):
    bf16 = ml_dtypes.bfloat16
    f32 = np.float32
    query = np.asarray(query, f32)
    key_value = np.asarray(key_value, f32)
    W_qkv = np.asarray(W_qkv, f32)
    b_qkv = np.asarray(b_qkv, f32)
    W_o = np.asarray(W_o, f32)
    b_o = np.asarray(b_o, f32)
    ln_gamma = np.asarray(ln_gamma, f32)
    ln_beta = np.asarray(ln_beta, f32)

    Wq, Wk, Wv = W_qkv[:D], W_qkv[D : 2 * D], W_qkv[2 * D :]
    bq, bk, bv = b_qkv[:D], b_qkv[D : 2 * D], b_qkv[2 * D :]

    woT_full = np.ascontiguousarray(W_o.T).astype(bf16)  # [d_in, n_out]
    gam = np.ascontiguousarray(np.broadcast_to(ln_gamma, (P, D))).astype(f32)
    bet = np.ascontiguousarray(np.broadcast_to(ln_beta, (P, D))).astype(f32)

    xqT = np.ascontiguousarray(query.transpose(0, 2, 1)).astype(bf16)
    xkvT = np.ascontiguousarray(key_value.transpose(0, 2, 1)).astype(bf16)

    in_maps = []
    for c in range(NCORES):
        b = c // GSZ
        jb = c % GSZ
        sl = slice(DLOC * c, DLOC * c + DLOC)  # this core's 2 heads
        # this core owns q rows 512*jc + 128*jb .. +128 of batch b
        res_rows = np.stack(
            [
                query[b, QCH * jc + P * jb : QCH * jc + P * jb + P] + b_o[None, :]
                for jc in range(NQC)
            ]
        )
        in_maps.append(
            {
                "xqT": xqT,
                "xkvT": xkvT,
                "wqT": np.ascontiguousarray(Wq[sl].T).astype(bf16),
                "wkT": np.ascontiguousarray(Wk[sl].T).astype(bf16),
                "wvT": np.ascontiguousarray(Wv[sl].T).astype(bf16),
                "bqs": (bq[sl] * 0.125)[:, None].astype(f32),
                "bks": bk[sl][:, None].astype(f32),
                "bvr": bv[sl][None, :].astype(bf16),
                "woT": woT_full,
                "qres": res_rows.astype(f32),
                "gam": gam,
                "bet": bet,
            }
        )
    return in_maps


def kernel(query, key_value, W_qkv, b_qkv, W_o, b_o, ln_gamma, ln_beta):
    global LAST_RESULT
    if "nc" not in _CACHE:
        _CACHE["nc"] = _build()
    nc = _CACHE["nc"]
    in_maps = _prep_inputs(
        query, key_value, W_qkv, b_qkv, W_o, b_o, ln_gamma, ln_beta
    )
    res = run_bass_kernel_spmd(nc, in_maps, core_ids=list(range(NCORES)))
    LAST_RESULT = res
    full = np.empty((B, SQ, D), np.float32)
    for c in range(NCORES):
        b = c // GSZ
        jb = c % GSZ
        o = res.results[c]["out"]  # [NQC, P, D]
        for jc in range(NQC):
            r0 = QCH * jc + P * jb
            full[b, r0 : r0 + P] = o[jc]
    return full


# revision 53
# speedup vs baseline: 1.0306x; 1.0306x over previous
"""Memory-efficient multi-head cross-attention on 8 TRN2 NeuronCores.

Sharding: each core owns 2 heads ({2c, 2c+1}) for BOTH batches
(tensor-parallel qkv projections over the head axis).  Per 512-row q chunk,
each core normalizes its context and a single dense 8-core AllToAll exchanges
[head-block x (batch, q-block)] tiles, after which every core holds the full
1024-channel context for its own (batch c//4, q rows 512*jc + 128*(c%4)).
The o-projection, residual add and LayerNorm then run fully locally -- no
cross-core reduction of o-proj partials is needed.

Pipeline (per chunk jc): attention(jc) emits Q'proj(jc+1), LN-stats(jc-2) and
o-proj(jc-1) as PE/DVE filler inside the (ACT-exp-bound) attention inner loop;
the A2A for chunk jc triggers as soon as both batches' context is normalized.
LayerNorm's Sqrt-dependent finish runs at the tail to avoid ACT table-set
switches between Exp and Sqrt.

kernel(**inputs) takes the FULL unsharded inputs and returns the FULL output.
"""

import sys
import types

import ml_dtypes
import numpy as np

# ---------------------------------------------------------------------------
# Environment shims (must run before concourse imports are used)
# ---------------------------------------------------------------------------


def _install_ntff_shim():
    """Provide antenv.axon_hooks (absent in this image) so that
    run_bass_kernel_spmd(trace=True) can capture NTFF profiles via the
    axon ctypes hook. Harmless when tracing is off."""
    if "antenv.axon_hooks" in sys.modules:
        return
    hook = None
    try:
        from trn_agent_boot.trn_boot import _ntff_profile_via_ctypes

        hook = _ntff_profile_via_ctypes("/opt/axon/libaxon_pjrt.so")
    except Exception:
        hook = None
    mod = types.ModuleType("antenv.axon_hooks")
    mod.get_axon_ntff_profile_hook = lambda: hook
    mod.set_axon_ntff_profile_hook = lambda h: None
    sys.modules["antenv.axon_hooks"] = mod


_install_ntff_shim()

import concourse.bass as bass  # noqa: E402
import concourse.mybir as mybir  # noqa: E402
import concourse.tile as tile  # noqa: E402
from concourse.bass_utils import run_bass_kernel_spmd  # noqa: E402
from concourse.vector_clock import ScopedClock  # noqa: E402
import bass_rust as _bass_rust  # noqa: E402


def _patched_drain_and_barrier(self, tick_clock, wait_clock):
    """The walrus build in this image rejects a Drain carrying multiple sem
    waits ("Too many sync wait commands").  Emit the kernel-tail waits as
    standalone wait instructions on the sync engine instead, then drain."""
    nc = self.nc
    probe = nc.sync.nop(nofuse=True)
    wait_clock.add_sem_waits(probe.ins, ScopedClock({None: tick_clock.global_clock}))
    waits = list(probe.ins.sync_info.on_wait)
    probe.ins.sync_info.on_wait.clear()
    name2sem = {s.name: s for s in self.sems.allocated().values()}
    for w in waits:
        nc.sync.wait_ge(name2sem[w.ant_name], w.wait_value)
    nc.sync.drain()
    nc.all_engine_barrier()
    popped = nc._tile_sem_poison_stack.pop()
    assert popped is self._sem_poison
    nc.clear_and_free_semaphores(list(self.sems.allocated().values()))
    nc.all_engine_barrier()


tile.TileContext._drain_and_barrier = _patched_drain_and_barrier

# Max sem-waits this walrus build accepts on a single instruction.
_WAIT_LIMIT = 1


def _split_waits(nc, limit=_WAIT_LIMIT):
    """Hoist excess per-instruction sem waits into standalone EventSemaphore
    instructions (same engine, immediately preceding), since this walrus build
    rejects instructions carrying more than one sync wait."""
    n_split = 0
    for f in nc.m.functions:
        for bb in f.blocks:
            insts = bb.instructions
            i = 0
            while i < len(insts):
                inst = insts[i]
                si = getattr(inst, "sync_info", None)
                waits = si.on_wait if si is not None else None
                if waits is not None and len(waits) > limit:
                    excess = list(waits)[limit:]
                    del waits[limit:]
                    for w in excess:
                        ev = mybir.InstEventSemaphore(
                            name=f"I-{nc.next_id()}",
                            engine=inst.engine,
                            ins=[],
                            outs=[],
                        )
                        ev.sync_info = mybir.SyncInfo(on_wait=[w], on_update=[])
                        insts.insert(i, ev)
                        i += 1
                        n_split += 1
                i += 1
    return n_split


# ---------------------------------------------------------------------------
# Problem constants (hardcoded per the harness contract)
# ---------------------------------------------------------------------------
B = 2
SQ = 2048
SKV = 2048
D = 1024
NH = 16
DK = 64

NCORES = 8
GSZ = 4  # q-blocks per chunk / cores per batch in the output mapping
DLOC = 2 * DK  # 128 local context channels (2 heads)
P = 128
QCH = 512  # q chunk (matmul moving free dim)
NQC = SQ // QCH  # 4
NKT = SKV // P  # 16 k tiles
NMT = D // P  # 8 contraction tiles over model dim

F32 = mybir.dt.float32
BF16 = mybir.dt.bfloat16

LN_EPS = 1e-5

_CACHE = {}
LAST_RESULT = None


def _build():
    """Build the SPMD Bass program (identical on all 8 cores)."""
    nc = bass.Bass("TRN2", target_bir_lowering=False, num_devices=NCORES)

    # ---- kernel I/O (per-core shards supplied by the host) ----
    # x inputs are full (both batches), transposed: [b, d, seq]
    xqT = nc.dram_tensor("xqT", [B, D, SQ], BF16, kind="ExternalInput")
    xkvT = nc.dram_tensor("xkvT", [B, D, SKV], BF16, kind="ExternalInput")
    # projection weights for this core's 2 heads: [d_in, 128]
    wqT = nc.dram_tensor("wqT", [D, DLOC], BF16, kind="ExternalInput")
    wkT = nc.dram_tensor("wkT", [D, DLOC], BF16, kind="ExternalInput")
    wvT = nc.dram_tensor("wvT", [D, DLOC], BF16, kind="ExternalInput")
    bqs = nc.dram_tensor("bqs", [P, 1], F32, kind="ExternalInput")
    bks = nc.dram_tensor("bks", [P, 1], F32, kind="ExternalInput")
    bvr = nc.dram_tensor("bvr", [1, DLOC], BF16, kind="ExternalInput")
    # full o-proj weights W_o.T [d_in, n]; rows 128*s.. = heads {2s, 2s+1}
    woT = nc.dram_tensor("woT", [D, D], BF16, kind="ExternalInput")
    # residual rows (query + b_o) for this core: [jc, 128, D]
    qres = nc.dram_tensor("qres", [NQC, P, D], F32, kind="ExternalInput")
    gam = nc.dram_tensor("gam", [P, D], F32, kind="ExternalInput")
    bet = nc.dram_tensor("bet", [P, D], F32, kind="ExternalInput")
    out = nc.dram_tensor("out", [NQC, P, D], F32, kind="ExternalOutput")

    groups = [[0, 1, 2, 3, 4, 5, 6, 7]]
    Exp = mybir.ActivationFunctionType.Exp
    Sqrt = mybir.ActivationFunctionType.Sqrt
    MULT = mybir.AluOpType.mult
    ADD = mybir.AluOpType.add

    xq_v = xqT.ap().rearrange("b (t p) q -> p b t q", p=P)
    xkv_v = xkvT.ap().rearrange("b (t p) q -> p b t q", p=P)

    with tile.TileContext(nc) as tc:
        with (
            tc.tile_pool(name="cpool", bufs=1) as cpool,
            tc.tile_pool(name="spool", bufs=2) as spool,
            tc.tile_pool(name="dram", bufs=1, space="DRAM") as dram,
        ):
            # ---- persistent SBUF tensors ----
            wq_sb = cpool.tile([P, NMT, DLOC], BF16)
            wk_sb = cpool.tile([P, NMT, DLOC], BF16)
            wv_sb = cpool.tile([P, NMT, DLOC], BF16)
            bqs_sb = cpool.tile([P, 1], F32)
            bks_sb = cpool.tile([P, 1], F32)
            bvr_sb = cpool.tile([1, DLOC], BF16)
            onesP = cpool.tile([P, P], BF16)
            eps_sb = cpool.tile([P, 1], F32)
            # K'^T per batch (d-of-head-pair on partitions)
            kt_sb = cpool.tile([P, B, SKV], BF16)
            v_sb = cpool.tile([P, B, NKT, DLOC], BF16)  # V rows (k on partitions)
            wo_sb = cpool.tile([P, NMT, D], BF16)  # W_o^T, slab s = heads {2s,2s+1}
            gam_sb = cpool.tile([P, D], F32)
            bet_sb = cpool.tile([P, D], F32)

            # Queue budget: sync+scalar carry the 8MB xkv stream (split); the
            # gpsimd queue issues the secondary loads as doorbells before the
            # warmup collective's completion wait blocks it.
            nc.sync.dma_start(wk_sb[:], wkT.ap().rearrange("(t p) d -> p t d", p=P))
            nc.sync.dma_start(bks_sb[:], bks.ap())
            nc.scalar.dma_start(wv_sb[:], wvT.ap().rearrange("(t p) d -> p t d", p=P))
            nc.scalar.dma_start(bvr_sb[:], bvr.ap())
            nc.vector.memset(onesP[:], 1.0)
            nc.vector.memset(eps_sb[:], LN_EPS)

            # xq chunk ring: [P, b, mt, q] for one chunk
            def xq_fetch(jc):
                xq_r = spool.tile(
                    [P, B, NMT, QCH], BF16, tag="xqr", name=f"xqr_{jc}", bufs=1
                )
                nc.sync.dma_start(
                    xq_r[:], xq_v[:, :, :, QCH * jc : QCH * jc + QCH]
                )
                return xq_r

            # residual rows ring: [P, D] for one chunk
            def qres_fetch(jc):
                qres_r = spool.tile(
                    [P, D], F32, tag="qres", name=f"qres_{jc}", bufs=3
                )
                nc.sync.dma_start(qres_r[:], qres.ap()[jc])
                return qres_r

            # ---------------- Phase A: K'/V projections ----------------
            # Warm up the collectives stream early: a tiny AllToAll during
            # phase A absorbs the ~30us first-collective ncfw startup cost
            # that would otherwise stall the chunk-1 o-proj pipeline.
            warm_sb = cpool.tile([P, NCORES, P], BF16)
            nc.vector.memset(warm_sb[:], 0.0)
            warm_in = dram.tile([NCORES * P, P], BF16, name="warm_in")
            warm_out = dram.tile([NCORES * P, P], BF16, name="warm_out")
            nc.gpsimd.dma_start(
                warm_in.rearrange("(s p) q -> p s q", p=P), warm_sb[:]
            )
            nc.gpsimd.collective_compute(
                "AllToAll",
                mybir.AluOpType.bypass,
                replica_groups=[[0, 1, 2, 3, 4, 5, 6, 7]],
                ins=[warm_in.opt()],
                outs=[warm_out.opt()],
            )

            xq_rings = {}
            with (
                tc.tile_pool(name="apool", bufs=1) as apool,
                tc.tile_pool(name="psA", bufs=8, space="PSUM") as psA,
            ):
                # spread the bulk xkv stream over two engines' hardware DMA
                # queues -- a single engine's queue tops out well below HBM
                # bandwidth
                xkv_tiles = []
                last_dma = {}
                for b in range(B):
                    xkv_sb = apool.tile(
                        [P, NMT, SKV], BF16, tag="xkv", name=f"xkv_{b}", bufs=2
                    )
                    xkv_tiles.append(xkv_sb)
                    for mt in range(NMT):
                        eng = "sync" if mt % 2 == 0 else "scalar"
                        last_dma[eng] = getattr(nc, eng).dma_start(
                            xkv_sb[:, mt, :], xkv_v[:, b, mt, :]
                        )
                # secondary loads behind the xkv stream, order pinned so the
                # scheduler cannot hoist their transfers ahead of it
                xq_rings[0] = xq_fetch(0)
                for dma in (
                    nc.scalar.dma_start(
                        wq_sb[:], wqT.ap().rearrange("(t p) d -> p t d", p=P)
                    ),
                    nc.scalar.dma_start(bqs_sb[:], bqs.ap()),
                    nc.scalar.dma_start(
                        wo_sb[:], woT.ap().rearrange("(t p) n -> p t n", p=P)
                    ),
                    nc.scalar.dma_start(gam_sb[:], gam.ap()),
                    nc.scalar.dma_start(bet_sb[:], bet.ap()),
                ):
                    _bass_rust.add_dep_helper(
                        dma.ins, last_dma["scalar"].ins, False, "dma order"
                    )
                for b in range(B):
                    xkv_sb = xkv_tiles[b]

                    # K'^T: out[dloc(128), kv(512)], m-tile outer so matmuls
                    # start as soon as the first input chunk lands.
                    pss = [
                        psA.tile([P, QCH], F32, tag="pj", name=f"pk_{b}_{i}")
                        for i in range(NQC)
                    ]
                    for mt in range(NMT):
                        for qc in range(NQC):
                            nc.tensor.matmul(
                                pss[qc][:],
                                lhsT=wk_sb[:, mt, :],
                                rhs=xkv_sb[:, mt, QCH * qc : QCH * qc + QCH],
                                start=(mt == 0),
                                stop=(mt == NMT - 1),
                            )
                    for qc in range(NQC):
                        nc.vector.tensor_scalar(
                            kt_sb[:, b, QCH * qc : QCH * qc + QCH],
                            pss[qc][:],
                            1.0,
                            bks_sb[:, 0:1],
                            MULT,
                            ADD,
                        )

                    # V: out[k_tile(128), dloc(128)] over m-tiles + bias row.
                    for kt in range(NKT):
                        ps = psA.tile([P, QCH], F32, tag="pj", name=f"pv_{b}_{kt}")
                        pv = ps[:, :DLOC]
                        for mt in range(NMT):
                            nc.tensor.matmul(
                                pv,
                                lhsT=xkv_sb[:, mt, P * kt : P * kt + P],
                                rhs=wv_sb[:, mt, :],
                                start=(mt == 0),
                                stop=False,
                            )
                        nc.tensor.matmul(
                            pv,
                            lhsT=onesP[0:1, :],
                            rhs=bvr_sb[0:1, :],
                            start=False,
                            stop=True,
                        )
                        nc.vector.tensor_copy(v_sb[:, b, kt, :], pv)

            # ------- Phase B: per-chunk attention + A2A + o-proj + LN -------
            with (
                tc.tile_pool(name="bpool", bufs=1) as bpool,
                tc.tile_pool(name="psB", bufs=1, space="PSUM") as psB,
            ):
                # exp(scores): [k(128), ktp, head*kt-in-pair, q]
                p_sb = bpool.tile([P, NKT // 2, 4, QCH], BF16)

                # The tile scheduler reorders instructions; filler steps must
                # be pinned behind the attention matmuls they are meant to
                # pad, or they get hoisted to the chunk start where their
                # input waits (collectives, DMAs) block the in-order queues.
                anchor = {}

                def pin(inst):
                    if "pe" in anchor:
                        _bass_rust.add_dep_helper(
                            inst.ins, anchor["pe"].ins, False, "filler order"
                        )
                    return inst

                cc_out = {}
                qt_rings = {}
                qres_rings = {}
                x_tiles = {}
                ln_st = {}

                def make_qproj_steps(jc):
                    """Q'^T projection for chunk jc: 4 filler steps."""
                    st = {}
                    qt_rings[jc] = spool.tile(
                        [P, B, QCH], BF16, tag="qt", name=f"qt_{jc}", bufs=2
                    )

                    def mm_step(b, half):
                        if half == 0:
                            st[b] = psB.tile(
                                [P, QCH], F32, tag="fill", name=f"pq_{jc}_{b}", bufs=1
                            )
                        pq = st[b]
                        xq_r = xq_rings[jc]
                        for mt in range(4 * half, 4 * half + 4):
                            pin(
                                nc.tensor.matmul(
                                    pq[:],
                                    lhsT=wq_sb[:, mt, :],
                                    rhs=xq_r[:, b, mt, :],
                                    start=(mt == 0),
                                    stop=(mt == NMT - 1),
                                )
                            )
                        if half == 1:
                            nc.vector.tensor_scalar(
                                qt_rings[jc][:, b, :],
                                pq[:],
                                0.125,
                                bqs_sb[:, 0:1],
                                MULT,
                                ADD,
                            )

                    return [
                        (lambda b=b, h=h: mm_step(b, h))
                        for b in range(B)
                        for h in range(2)
                    ]

                def make_oproj_steps(jc):
                    """A2A receive + o-projection for chunk jc: 5 steps."""
                    st = {}

                    def dmain_step():
                        # on the scalar queue: its HWDGE doorbell is ~10x
                        # cheaper than gpsimd's SWDGE descriptor build, and
                        # the cc_out it waits on is a 2-chunk-old collective,
                        # so it never stalls the exp stream behind it.
                        ctg = spool.tile(
                            [P, NCORES, P], BF16, tag="ctg", name=f"ctg_{jc}", bufs=2
                        )
                        st["ctg"] = ctg
                        pin(
                            nc.scalar.dma_start(
                                ctg[:], cc_out[jc].rearrange("(s p) q -> p s q", p=P)
                            )
                        )

                    def mm_step(nch, half):
                        if nch == 0 and half == 0:
                            x_tiles[jc] = spool.tile(
                                [P, D], F32, tag="x", name=f"x_{jc}", bufs=2
                            )
                        if half == 0:
                            st["po"] = psB.tile(
                                [P, QCH], F32, tag="fill", name=f"po_{jc}_{nch}", bufs=1
                            )
                        po = st["po"]
                        ctg = st["ctg"]
                        nsl = slice(QCH * nch, QCH * nch + QCH)
                        for s in range(4 * half, 4 * half + 4):
                            pin(
                                nc.tensor.matmul(
                                    po[:],
                                    lhsT=ctg[:, s],
                                    rhs=wo_sb[:, s, nsl],
                                    start=(s == 0),
                                    stop=(s == NCORES - 1),
                                )
                            )
                        if half == 1:
                            nc.vector.tensor_add(
                                x_tiles[jc][:, nsl], po[:], qres_rings[jc][:, nsl]
                            )

                    return [dmain_step] + [
                        (lambda n=n, h=h: mm_step(n, h))
                        for n in range(2)
                        for h in range(2)
                    ]

                def make_ln_stat_steps(jc):
                    """LayerNorm statistics for chunk jc (DVE only): 2 steps."""
                    st = {}
                    ln_st[jc] = st

                    def s1():
                        x = x_tiles[jc]
                        mean = spool.tile(
                            [P, 1], F32, tag="mean", name=f"mean_{jc}", bufs=4
                        )
                        nmean = spool.tile(
                            [P, 1], F32, tag="nmean", name=f"nmean_{jc}", bufs=4
                        )
                        xc = spool.tile([P, D], F32, tag="xc", name=f"xc_{jc}", bufs=3)
                        st["xc"] = xc
                        pin(
                            nc.vector.reduce_sum(
                                mean[:], x[:], axis=mybir.AxisListType.X
                            )
                        )
                        nc.vector.tensor_scalar_mul(nmean[:], mean[:], -1.0 / D)
                        nc.vector.tensor_scalar_add(xc[:], x[:], nmean[:])

                    def s2():
                        xc = st["xc"]
                        sq = spool.tile([P, D], F32, tag="sq", name=f"sq_{jc}", bufs=1)
                        ssq = spool.tile(
                            [P, 1], F32, tag="ssq", name=f"ssq_{jc}", bufs=4
                        )
                        st["ssq"] = ssq
                        nc.vector.tensor_mul(sq[:], xc[:], xc[:])
                        nc.vector.reduce_sum(ssq[:], sq[:], axis=mybir.AxisListType.X)

                    return [s1, s2]

                def ln_finish(jc):
                    """Sqrt + scale/shift + output DMA for chunk jc (tail)."""
                    xc = ln_st[jc]["xc"]
                    ssq = ln_st[jc]["ssq"]
                    sd = spool.tile([P, 1], F32, tag="sd", name=f"sd_{jc}", bufs=2)
                    rstd = spool.tile(
                        [P, 1], F32, tag="rstd", name=f"rstd_{jc}", bufs=2
                    )
                    y = spool.tile([P, D], F32, tag="y", name=f"y_{jc}", bufs=2)
                    nc.scalar.activation(
                        sd[:], ssq[:], Sqrt, scale=1.0 / D, bias=eps_sb[:]
                    )
                    nc.vector.reciprocal(rstd[:], sd[:])
                    nc.vector.tensor_scalar_mul(y[:], xc[:], rstd[:])
                    nc.vector.tensor_mul(y[:], y[:], gam_sb[:])
                    nc.vector.tensor_add(y[:], y[:], bet_sb[:])
                    nc.sync.dma_start(out.ap()[jc], y[:])

                def attn_chunk(jc, fillers):
                    # prefetch next chunk's x_q slice + this chunk's residual
                    if jc + 1 < NQC:
                        xq_rings[jc + 1] = xq_fetch(jc + 1)
                    qres_rings[jc] = qres_fetch(jc)
                    ct = spool.tile(
                        [P, B, GSZ, P], BF16, tag="ct", name=f"ct_{jc}", bufs=2
                    )
                    cc_in = dram.tile([NCORES * P, P], BF16, name=f"cc_in_{jc}")
                    cc_in_v = cc_in.rearrange("(d p) q -> p d q", p=P)
                    qt_r = qt_rings[jc]
                    for b in range(B):
                        ctx = psB.tile(
                            [P, QCH], F32, tag="ctx", name=f"ctx_{jc}_{b}", bufs=2
                        )
                        dn = psB.tile(
                            [P, QCH], F32, tag="dn", name=f"dn_{jc}_{b}", bufs=1
                        )

                        def scores(kt):
                            # one kt (both heads) per 2-bank psum slab; slabs
                            # ping-pong (tags sA/sB) so the next kt's matmuls
                            # run while ACT still exps the previous slab.
                            j = kt % 2
                            s2 = psB.tile(
                                [P, 2, QCH],
                                F32,
                                tag=("sA", "sB")[j],
                                name=f"s_{jc}_{b}_{kt}",
                                bufs=1,
                            )
                            ksl = slice(P * kt, P * kt + P)
                            nc.tensor.matmul(
                                s2[:, 0, :],
                                lhsT=kt_sb[0:DK, b, ksl],
                                rhs=qt_r[0:DK, b, :],
                            )
                            anchor["pe"] = nc.tensor.matmul(
                                s2[:, 1, :],
                                lhsT=kt_sb[DK:P, b, ksl],
                                rhs=qt_r[DK:P, b, :],
                            )
                            nc.scalar.activation(
                                p_sb[:, kt // 2, 2 * j : 2 * j + 2], s2[:], Exp
                            )

                        def ctx2(kt):
                            ktp, j = kt // 2, kt % 2
                            st_, sp = kt == 0, kt == NKT - 1
                            nc.tensor.matmul(
                                ctx[0:DK, :],
                                lhsT=v_sb[:, b, kt, 0:DK],
                                rhs=p_sb[:, ktp, 2 * j, :],
                                start=st_,
                                stop=sp,
                            )
                            nc.tensor.matmul(
                                ctx[DK:P, :],
                                lhsT=v_sb[:, b, kt, DK:P],
                                rhs=p_sb[:, ktp, 2 * j + 1, :],
                                start=st_,
                                stop=sp,
                            )
                            # denominator rows, broadcast across the head's 64
                            # partitions by the ones stationary (same
                            # 512-cycle stream as a 1-row output)
                            nc.tensor.matmul(
                                dn[0:DK, :],
                                lhsT=onesP[:, 0:DK],
                                rhs=p_sb[:, ktp, 2 * j, :],
                                start=st_,
                                stop=sp,
                            )
                            anchor["pe"] = nc.tensor.matmul(
                                dn[DK:P, :],
                                lhsT=onesP[:, DK:P],
                                rhs=p_sb[:, ktp, 2 * j + 1, :],
                                start=st_,
                                stop=sp,
                            )

                        # fillers pop between the scores pair (which feeds the
                        # ACT exp stream) and the ctx/dn accumulation, so they
                        # bridge the boundary reciprocal-WAR stall on dn
                        # without delaying the next exp.
                        scores(0)
                        scores(1)
                        if b == 1:
                            for _ in range(2):
                                if fillers:
                                    fillers.pop(0)()
                        for kt2 in range(2, NKT, 2):
                            scores(kt2)
                            scores(kt2 + 1)
                            if (kt2 >= 4 or kt2 == 2) and fillers:
                                fillers.pop(0)()
                            ctx2(kt2 - 2)
                            ctx2(kt2 - 1)
                        ctx2(NKT - 2)
                        ctx2(NKT - 1)

                        # normalize: ct = ctx * (1/dn), then ship to DRAM for
                        # the A2A (dn rows are already broadcast per head)
                        rd = spool.tile(
                            [P, QCH], F32, tag="rd", name=f"rd_{jc}_{b}", bufs=2
                        )
                        nc.vector.reciprocal(rd[:], dn[:])
                        nc.vector.tensor_mul(ct[:, b], ctx[:], rd[:])
                        nc.sync.dma_start(
                            cc_in_v[:, GSZ * b : GSZ * b + GSZ, :], ct[:, b]
                        )

                    cc_out[jc] = dram.tile([NCORES * P, P], BF16, name=f"cc_out_{jc}")
                    nc.gpsimd.collective_compute(
                        "AllToAll",
                        mybir.AluOpType.bypass,
                        replica_groups=groups,
                        ins=[cc_in.opt()],
                        outs=[cc_out[jc].opt()],
                    )

                    while fillers:
                        fillers.pop(0)()

                # Q'(0) before the first chunk
                for step in make_qproj_steps(0):
                    step()

                def make_warm_steps(jc, n):
                    # spare PE work for thin early chunks: keeps HAM at full
                    # clock across the softmax-reciprocal boundary stalls
                    def wstep(i):
                        wm = psB.tile(
                            [P, QCH], F32, tag="fill", name=f"wf_{jc}_{i}", bufs=1
                        )
                        for _ in range(2):
                            pin(
                                nc.tensor.matmul(
                                    wm[:], lhsT=onesP[:], rhs=wo_sb[:, 0, 0:QCH]
                                )
                            )

                    return [(lambda i=i: wstep(i)) for i in range(n)]

                for jc in range(NQC):
                    # o-proj for chunk jc-2: its A2A has had a full chunk to
                    # complete, so these steps never stall the PE queue even
                    # when a collective runs long.
                    fillers = []
                    if jc >= 2:
                        fillers += make_oproj_steps(jc - 2)
                    else:
                        # early pops must not touch the just-issued xq DMA
                        fillers += make_warm_steps(jc, 2)
                    if jc < NQC - 1:
                        fillers += make_qproj_steps(jc + 1)
                    if jc >= 2:
                        fillers += make_ln_stat_steps(jc - 2)
                    else:
                        fillers += make_warm_steps(jc + 10, 3)
                    attn_chunk(jc, fillers)

                # ---- tail ----
                # chunk 2's o-proj (A2A long done) + LN finishes overlap the
                # last chunk's A2A; keep-warm matmuls hold the PE at full
                # clock across the collective wait so o-proj(3) runs warm.
                ln_finish(0)
                ln_finish(1)
                for step in make_oproj_steps(2):
                    step()
                for step in make_ln_stat_steps(2):
                    step()
                ln_finish(2)
                for i in range(28):
                    wm = psB.tile(
                        [P, QCH], F32, tag="fill", name=f"warmmm_{i}", bufs=1
                    )
                    pin(
                        nc.tensor.matmul(
                            wm[:], lhsT=onesP[:], rhs=wo_sb[:, 0, 0:QCH]
                        )
                    )
                for step in make_oproj_steps(NQC - 1):
                    step()
                for step in make_ln_stat_steps(NQC - 1):
                    step()
                ln_finish(NQC - 1)

    _split_waits(nc)
    return nc


def _prep_inputs(query, key_value, W_qkv, b_qkv, W_o, b_o, ln_gamma, ln_beta):
    bf16 = ml_dtypes.bfloat16
    f32 = np.float32
    query = np.asarray(query, f32)
    key_value = np.asarray(key_value, f32)
    W_qkv = np.asarray(W_qkv, f32)
    b_qkv = np.asarray(b_qkv, f32)
    W_o = np.asarray(W_o, f32)
    b_o = np.asarray(b_o, f32)
    ln_gamma = np.asarray(ln_gamma, f32)
    ln_beta = np.asarray(ln_beta, f32)

    Wq, Wk, Wv = W_qkv[:D], W_qkv[D : 2 * D], W_qkv[2 * D :]
    bq, bk, bv = b_qkv[:D], b_qkv[D : 2 * D], b_qkv[2 * D :]

    woT_full = np.ascontiguousarray(W_o.T).astype(bf16)  # [d_in, n_out]
    gam = np.ascontiguousarray(np.broadcast_to(ln_gamma, (P, D))).astype(f32)
    bet = np.ascontiguousarray(np.broadcast_to(ln_beta, (P, D))).astype(f32)

    xqT = np.ascontiguousarray(query.transpose(0, 2, 1)).astype(bf16)
    xkvT = np.ascontiguousarray(key_value.transpose(0, 2, 1)).astype(bf16)

    in_maps = []
    for c in range(NCORES):
        b = c // GSZ
        jb = c % GSZ
        sl = slice(DLOC * c, DLOC * c + DLOC)  # this core's 2 heads
        # this core owns q rows 512*jc + 128*jb .. +128 of batch b
        res_rows = np.stack(
            [
                query[b, QCH * jc + P * jb : QCH * jc + P * jb + P] + b_o[None, :]
                for jc in range(NQC)
            ]
        )
        in_maps.append(
            {
                "xqT": xqT,
                "xkvT": xkvT,
                "wqT": np.ascontiguousarray(Wq[sl].T).astype(bf16),
                "wkT": np.ascontiguousarray(Wk[sl].T).astype(bf16),
                "wvT": np.ascontiguousarray(Wv[sl].T).astype(bf16),
                "bqs": (bq[sl] * 0.125)[:, None].astype(f32),
                "bks": bk[sl][:, None].astype(f32),
                "bvr": bv[sl][None, :].astype(bf16),
                "woT": woT_full,
                "qres": res_rows.astype(f32),
                "gam": gam,
                "bet": bet,
            }
        )
    return in_maps


def kernel(query, key_value, W_qkv, b_qkv, W_o, b_o, ln_gamma, ln_beta):
    global LAST_RESULT
    if "nc" not in _CACHE:
        _CACHE["nc"] = _build()
    nc = _CACHE["nc"]
    in_maps = _prep_inputs(
        query, key_value, W_qkv, b_qkv, W_o, b_o, ln_gamma, ln_beta
    )
    res = run_bass_kernel_spmd(nc, in_maps, core_ids=list(range(NCORES)))
    LAST_RESULT = res
    full = np.empty((B, SQ, D), np.float32)
    for c in range(NCORES):
        b = c // GSZ
        jb = c % GSZ
        o = res.results[c]["out"]  # [NQC, P, D]
        for jc in range(NQC):
            r0 = QCH * jc + P * jb
            full[b, r0 : r0 + P] = o[jc]
    return full


# revision 54
# speedup vs baseline: 1.0594x; 1.0279x over previous
"""Memory-efficient multi-head cross-attention on 8 TRN2 NeuronCores.

Sharding: each core owns 2 heads ({2c, 2c+1}) for BOTH batches
(tensor-parallel qkv projections over the head axis).  Per 512-row q chunk,
each core normalizes its context and a single dense 8-core AllToAll exchanges
[head-block x (batch, q-block)] tiles, after which every core holds the full
1024-channel context for its own (batch c//4, q rows 512*jc + 128*(c%4)).
The o-projection, residual add and LayerNorm then run fully locally -- no
cross-core reduction of o-proj partials is needed.

Pipeline (per chunk jc): attention(jc) emits Q'proj(jc+1), LN-stats(jc-2) and
o-proj(jc-1) as PE/DVE filler inside the (ACT-exp-bound) attention inner loop;
the A2A for chunk jc triggers as soon as both batches' context is normalized.
LayerNorm's Sqrt-dependent finish runs at the tail to avoid ACT table-set
switches between Exp and Sqrt.

kernel(**inputs) takes the FULL unsharded inputs and returns the FULL output.
"""

import sys
import types

import ml_dtypes
import numpy as np

# ---------------------------------------------------------------------------
# Environment shims (must run before concourse imports are used)
# ---------------------------------------------------------------------------


def _install_ntff_shim():
    """Provide antenv.axon_hooks (absent in this image) so that
    run_bass_kernel_spmd(trace=True) can capture NTFF profiles via the
    axon ctypes hook. Harmless when tracing is off."""
    if "antenv.axon_hooks" in sys.modules:
        return
    hook = None
    try:
        from trn_agent_boot.trn_boot import _ntff_profile_via_ctypes

        hook = _ntff_profile_via_ctypes("/opt/axon/libaxon_pjrt.so")
    except Exception:
        hook = None
    mod = types.ModuleType("antenv.axon_hooks")
    mod.get_axon_ntff_profile_hook = lambda: hook
    mod.set_axon_ntff_profile_hook = lambda h: None
    sys.modules["antenv.axon_hooks"] = mod


_install_ntff_shim()

import concourse.bass as bass  # noqa: E402
import concourse.mybir as mybir  # noqa: E402
import concourse.tile as tile  # noqa: E402
from concourse.bass_utils import run_bass_kernel_spmd  # noqa: E402
from concourse.vector_clock import ScopedClock  # noqa: E402
import bass_rust as _bass_rust  # noqa: E402


def _patched_drain_and_barrier(self, tick_clock, wait_clock):
    """The walrus build in this image rejects a Drain carrying multiple sem
    waits ("Too many sync wait commands").  Emit the kernel-tail waits as
    standalone wait instructions on the sync engine instead, then drain."""
    nc = self.nc
    probe = nc.sync.nop(nofuse=True)
    wait_clock.add_sem_waits(probe.ins, ScopedClock({None: tick_clock.global_clock}))
    waits = list(probe.ins.sync_info.on_wait)
    probe.ins.sync_info.on_wait.clear()
    name2sem = {s.name: s for s in self.sems.allocated().values()}
    for w in waits:
        nc.sync.wait_ge(name2sem[w.ant_name], w.wait_value)
    nc.sync.drain()
    nc.all_engine_barrier()
    popped = nc._tile_sem_poison_stack.pop()
    assert popped is self._sem_poison
    nc.clear_and_free_semaphores(list(self.sems.allocated().values()))
    nc.all_engine_barrier()


tile.TileContext._drain_and_barrier = _patched_drain_and_barrier

# Max sem-waits this walrus build accepts on a single instruction.
_WAIT_LIMIT = 1


def _split_waits(nc, limit=_WAIT_LIMIT):
    """Hoist excess per-instruction sem waits into standalone EventSemaphore
    instructions (same engine, immediately preceding), since this walrus build
    rejects instructions carrying more than one sync wait."""
    n_split = 0
    for f in nc.m.functions:
        for bb in f.blocks:
            insts = bb.instructions
            i = 0
            while i < len(insts):
                inst = insts[i]
                si = getattr(inst, "sync_info", None)
                waits = si.on_wait if si is not None else None
                if waits is not None and len(waits) > limit:
                    excess = list(waits)[limit:]
                    del waits[limit:]
                    for w in excess:
                        ev = mybir.InstEventSemaphore(
                            name=f"I-{nc.next_id()}",
                            engine=inst.engine,
                            ins=[],
                            outs=[],
                        )
                        ev.sync_info = mybir.SyncInfo(on_wait=[w], on_update=[])
                        insts.insert(i, ev)
                        i += 1
                        n_split += 1
                i += 1
    return n_split


# ---------------------------------------------------------------------------
# Problem constants (hardcoded per the harness contract)
# ---------------------------------------------------------------------------
B = 2
SQ = 2048
SKV = 2048
D = 1024
NH = 16
DK = 64

NCORES = 8
GSZ = 4  # q-blocks per chunk / cores per batch in the output mapping
DLOC = 2 * DK  # 128 local context channels (2 heads)
P = 128
QCH = 512  # q chunk (matmul moving free dim)
NQC = SQ // QCH  # 4
NKT = SKV // P  # 16 k tiles
NMT = D // P  # 8 contraction tiles over model dim

F32 = mybir.dt.float32
BF16 = mybir.dt.bfloat16

LN_EPS = 1e-5

_CACHE = {}
LAST_RESULT = None


def _build():
    """Build the SPMD Bass program (identical on all 8 cores)."""
    nc = bass.Bass("TRN2", target_bir_lowering=False, num_devices=NCORES)

    # ---- kernel I/O (per-core shards supplied by the host) ----
    # x inputs are full (both batches), transposed: [b, d, seq]
    xqT = nc.dram_tensor("xqT", [B, D, SQ], BF16, kind="ExternalInput")
    xkvT = nc.dram_tensor("xkvT", [B, D, SKV], BF16, kind="ExternalInput")
    # projection weights for this core's 2 heads: [d_in, 128]
    wqT = nc.dram_tensor("wqT", [D, DLOC], BF16, kind="ExternalInput")
    wkT = nc.dram_tensor("wkT", [D, DLOC], BF16, kind="ExternalInput")
    wvT = nc.dram_tensor("wvT", [D, DLOC], BF16, kind="ExternalInput")
    bqs = nc.dram_tensor("bqs", [P, 1], F32, kind="ExternalInput")
    bks = nc.dram_tensor("bks", [P, 1], F32, kind="ExternalInput")
    bvr = nc.dram_tensor("bvr", [1, DLOC], BF16, kind="ExternalInput")
    # full o-proj weights W_o.T [d_in, n]; rows 128*s.. = heads {2s, 2s+1}
    woT = nc.dram_tensor("woT", [D, D], BF16, kind="ExternalInput")
    # residual rows (query + b_o) for this core: [jc, 128, D]
    qres = nc.dram_tensor("qres", [NQC, P, D], F32, kind="ExternalInput")
    gam = nc.dram_tensor("gam", [P, D], F32, kind="ExternalInput")
    bet = nc.dram_tensor("bet", [P, D], F32, kind="ExternalInput")
    out = nc.dram_tensor("out", [NQC, P, D], F32, kind="ExternalOutput")

    groups = [[0, 1, 2, 3, 4, 5, 6, 7]]
    Exp = mybir.ActivationFunctionType.Exp
    Sqrt = mybir.ActivationFunctionType.Sqrt
    MULT = mybir.AluOpType.mult
    ADD = mybir.AluOpType.add

    xq_v = xqT.ap().rearrange("b (t p) q -> p b t q", p=P)
    xkv_v = xkvT.ap().rearrange("b (t p) q -> p b t q", p=P)

    with tile.TileContext(nc) as tc:
        with (
            tc.tile_pool(name="cpool", bufs=1) as cpool,
            tc.tile_pool(name="spool", bufs=2) as spool,
            tc.tile_pool(name="dram", bufs=1, space="DRAM") as dram,
        ):
            # ---- persistent SBUF tensors ----
            wq_sb = cpool.tile([P, NMT, DLOC], BF16)
            wk_sb = cpool.tile([P, NMT, DLOC], BF16)
            wv_sb = cpool.tile([P, NMT, DLOC], BF16)
            bqs_sb = cpool.tile([P, 1], F32)
            bks_sb = cpool.tile([P, 1], F32)
            bvr_sb = cpool.tile([1, DLOC], BF16)
            onesP = cpool.tile([P, P], BF16)
            eps_sb = cpool.tile([P, 1], F32)
            # K'^T per batch (d-of-head-pair on partitions)
            kt_sb = cpool.tile([P, B, SKV], BF16)
            v_sb = cpool.tile([P, B, NKT, DLOC], BF16)  # V rows (k on partitions)
            wo_sb = cpool.tile([P, NMT, D], BF16)  # W_o^T, slab s = heads {2s,2s+1}
            gam_sb = cpool.tile([P, D], F32)
            bet_sb = cpool.tile([P, D], F32)

            # Queue budget: sync+scalar carry the 8MB xkv stream (split); the
            # gpsimd queue issues the secondary loads as doorbells before the
            # warmup collective's completion wait blocks it.
            nc.sync.dma_start(wk_sb[:], wkT.ap().rearrange("(t p) d -> p t d", p=P))
            nc.sync.dma_start(bks_sb[:], bks.ap())
            nc.scalar.dma_start(wv_sb[:], wvT.ap().rearrange("(t p) d -> p t d", p=P))
            nc.scalar.dma_start(bvr_sb[:], bvr.ap())
            nc.vector.memset(onesP[:], 1.0)
            nc.vector.memset(eps_sb[:], LN_EPS)

            # xq chunk ring: [P, b, mt, q] for one chunk
            def xq_fetch(jc):
                xq_r = spool.tile(
                    [P, B, NMT, QCH], BF16, tag="xqr", name=f"xqr_{jc}", bufs=1
                )
                nc.sync.dma_start(
                    xq_r[:], xq_v[:, :, :, QCH * jc : QCH * jc + QCH]
                )
                return xq_r

            # residual rows ring: [P, D] for one chunk
            def qres_fetch(jc):
                qres_r = spool.tile(
                    [P, D], F32, tag="qres", name=f"qres_{jc}", bufs=3
                )
                nc.sync.dma_start(qres_r[:], qres.ap()[jc])
                return qres_r

            # ---------------- Phase A: K'/V projections ----------------
            # Warm up the collectives stream early: a tiny AllToAll during
            # phase A absorbs the ~30us first-collective ncfw startup cost
            # that would otherwise stall the chunk-1 o-proj pipeline.
            warm_sb = cpool.tile([P, NCORES, P], BF16)
            nc.vector.memset(warm_sb[:], 0.0)
            warm_in = dram.tile([NCORES * P, P], BF16, name="warm_in")
            warm_out = dram.tile([NCORES * P, P], BF16, name="warm_out")
            nc.gpsimd.dma_start(
                warm_in.rearrange("(s p) q -> p s q", p=P), warm_sb[:]
            )
            nc.gpsimd.collective_compute(
                "AllToAll",
                mybir.AluOpType.bypass,
                replica_groups=[[0, 1, 2, 3, 4, 5, 6, 7]],
                ins=[warm_in.opt()],
                outs=[warm_out.opt()],
            )

            xq_rings = {}
            with (
                tc.tile_pool(name="apool", bufs=1) as apool,
                tc.tile_pool(name="psA", bufs=8, space="PSUM") as psA,
            ):
                # spread the bulk xkv stream over two engines' hardware DMA
                # queues -- a single engine's queue tops out well below HBM
                # bandwidth
                xkv_tiles = []
                last_dma = {}
                for b in range(B):
                    xkv_sb = apool.tile(
                        [P, NMT, SKV], BF16, tag="xkv", name=f"xkv_{b}", bufs=2
                    )
                    xkv_tiles.append(xkv_sb)
                    for mt in range(NMT):
                        eng = "sync" if mt % 2 == 0 else "scalar"
                        last_dma[eng] = getattr(nc, eng).dma_start(
                            xkv_sb[:, mt, :], xkv_v[:, b, mt, :]
                        )
                # secondary loads behind the xkv stream, order pinned so the
                # scheduler cannot hoist their transfers ahead of it
                xq_rings[0] = xq_fetch(0)
                for dma in (
                    nc.scalar.dma_start(
                        wq_sb[:], wqT.ap().rearrange("(t p) d -> p t d", p=P)
                    ),
                    nc.scalar.dma_start(bqs_sb[:], bqs.ap()),
                    nc.scalar.dma_start(
                        wo_sb[:], woT.ap().rearrange("(t p) n -> p t n", p=P)
                    ),
                    nc.scalar.dma_start(gam_sb[:], gam.ap()),
                    nc.scalar.dma_start(bet_sb[:], bet.ap()),
                ):
                    _bass_rust.add_dep_helper(
                        dma.ins, last_dma["scalar"].ins, False, "dma order"
                    )
                for b in range(B):
                    xkv_sb = xkv_tiles[b]

                    # K'^T: out[dloc(128), kv(512)], m-tile outer so matmuls
                    # start as soon as the first input chunk lands.
                    pss = [
                        psA.tile([P, QCH], F32, tag="pj", name=f"pk_{b}_{i}")
                        for i in range(NQC)
                    ]
                    for mt in range(NMT):
                        for qc in range(NQC):
                            nc.tensor.matmul(
                                pss[qc][:],
                                lhsT=wk_sb[:, mt, :],
                                rhs=xkv_sb[:, mt, QCH * qc : QCH * qc + QCH],
                                start=(mt == 0),
                                stop=(mt == NMT - 1),
                            )
                    for qc in range(NQC):
                        nc.vector.tensor_scalar(
                            kt_sb[:, b, QCH * qc : QCH * qc + QCH],
                            pss[qc][:],
                            1.0,
                            bks_sb[:, 0:1],
                            MULT,
                            ADD,
                        )

                    # V: out[k_tile(128), dloc(128)] over m-tiles + bias row.
                    for kt in range(NKT):
                        ps = psA.tile([P, QCH], F32, tag="pj", name=f"pv_{b}_{kt}")
                        pv = ps[:, :DLOC]
                        for mt in range(NMT):
                            nc.tensor.matmul(
                                pv,
                                lhsT=xkv_sb[:, mt, P * kt : P * kt + P],
                                rhs=wv_sb[:, mt, :],
                                start=(mt == 0),
                                stop=False,
                            )
                        nc.tensor.matmul(
                            pv,
                            lhsT=onesP[0:1, :],
                            rhs=bvr_sb[0:1, :],
                            start=False,
                            stop=True,
                        )
                        nc.vector.tensor_copy(v_sb[:, b, kt, :], pv)

            # ------- Phase B: per-chunk attention + A2A + o-proj + LN -------
            with (
                tc.tile_pool(name="bpool", bufs=1) as bpool,
                tc.tile_pool(name="psB", bufs=1, space="PSUM") as psB,
            ):
                # exp(scores): [k(128), ktp, head*kt-in-pair, q]
                p_sb = bpool.tile([P, NKT // 2, 4, QCH], BF16)

                # The tile scheduler reorders instructions; filler steps must
                # be pinned behind the attention matmuls they are meant to
                # pad, or they get hoisted to the chunk start where their
                # input waits (collectives, DMAs) block the in-order queues.
                anchor = {}

                def pin(inst):
                    if "pe" in anchor:
                        _bass_rust.add_dep_helper(
                            inst.ins, anchor["pe"].ins, False, "filler order"
                        )
                    return inst

                cc_out = {}
                qt_rings = {}
                qres_rings = {}
                x_tiles = {}
                ln_st = {}

                def make_qproj_steps(jc):
                    """Q'^T projection for chunk jc: 4 filler steps."""
                    st = {}
                    qt_rings[jc] = spool.tile(
                        [P, B, QCH], BF16, tag="qt", name=f"qt_{jc}", bufs=2
                    )

                    def mm_step(b, half):
                        if half == 0:
                            st[b] = psB.tile(
                                [P, QCH], F32, tag="fill", name=f"pq_{jc}_{b}", bufs=1
                            )
                        pq = st[b]
                        xq_r = xq_rings[jc]
                        for mt in range(4 * half, 4 * half + 4):
                            pin(
                                nc.tensor.matmul(
                                    pq[:],
                                    lhsT=wq_sb[:, mt, :],
                                    rhs=xq_r[:, b, mt, :],
                                    start=(mt == 0),
                                    stop=(mt == NMT - 1),
                                )
                            )
                        if half == 1:
                            nc.vector.tensor_scalar(
                                qt_rings[jc][:, b, :],
                                pq[:],
                                0.125,
                                bqs_sb[:, 0:1],
                                MULT,
                                ADD,
                            )

                    return [
                        (lambda b=b, h=h: mm_step(b, h))
                        for b in range(B)
                        for h in range(2)
                    ]

                def make_oproj_steps(jc):
                    """A2A receive + o-projection for chunk jc: 5 steps."""
                    st = {}

                    def dmain_step():
                        # Mid-stream (jc<=1): scalar queue -- its HWDGE
                        # doorbell is ~10x cheaper than gpsimd's SWDGE build,
                        # and the cc_out it waits on is a 2-chunk-old
                        # collective, so it never stalls the exps behind it.
                        # Tail (jc>=2): gpsimd queue -- the last A2A is still
                        # in flight there, and a scalar-queue doorbell waiting
                        # on it would block the LayerNorm sqrts behind it.
                        ctg = spool.tile(
                            [P, NCORES, P], BF16, tag="ctg", name=f"ctg_{jc}", bufs=2
                        )
                        st["ctg"] = ctg
                        if jc >= 2:
                            nc.gpsimd.dma_start(
                                ctg[:], cc_out[jc].rearrange("(s p) q -> p s q", p=P)
                            )
                        else:
                            pin(
                                nc.scalar.dma_start(
                                    ctg[:],
                                    cc_out[jc].rearrange("(s p) q -> p s q", p=P),
                                )
                            )

                    def mm_step(nch, half):
                        if nch == 0 and half == 0:
                            x_tiles[jc] = spool.tile(
                                [P, D], F32, tag="x", name=f"x_{jc}", bufs=2
                            )
                        if half == 0:
                            st["po"] = psB.tile(
                                [P, QCH], F32, tag="fill", name=f"po_{jc}_{nch}", bufs=1
                            )
                        po = st["po"]
                        ctg = st["ctg"]
                        nsl = slice(QCH * nch, QCH * nch + QCH)
                        for s in range(4 * half, 4 * half + 4):
                            pin(
                                nc.tensor.matmul(
                                    po[:],
                                    lhsT=ctg[:, s],
                                    rhs=wo_sb[:, s, nsl],
                                    start=(s == 0),
                                    stop=(s == NCORES - 1),
                                )
                            )
                        if half == 1:
                            nc.vector.tensor_add(
                                x_tiles[jc][:, nsl], po[:], qres_rings[jc][:, nsl]
                            )

                    return [dmain_step] + [
                        (lambda n=n, h=h: mm_step(n, h))
                        for n in range(2)
                        for h in range(2)
                    ]

                def make_ln_stat_steps(jc):
                    """LayerNorm statistics for chunk jc (DVE only): 2 steps."""
                    st = {}
                    ln_st[jc] = st

                    def s1():
                        x = x_tiles[jc]
                        mean = spool.tile(
                            [P, 1], F32, tag="mean", name=f"mean_{jc}", bufs=4
                        )
                        nmean = spool.tile(
                            [P, 1], F32, tag="nmean", name=f"nmean_{jc}", bufs=4
                        )
                        xc = spool.tile([P, D], F32, tag="xc", name=f"xc_{jc}", bufs=3)
                        st["xc"] = xc
                        pin(
                            nc.vector.reduce_sum(
                                mean[:], x[:], axis=mybir.AxisListType.X
                            )
                        )
                        nc.vector.tensor_scalar_mul(nmean[:], mean[:], -1.0 / D)
                        nc.vector.tensor_scalar_add(xc[:], x[:], nmean[:])

                    def s2():
                        xc = st["xc"]
                        sq = spool.tile([P, D], F32, tag="sq", name=f"sq_{jc}", bufs=1)
                        ssq = spool.tile(
                            [P, 1], F32, tag="ssq", name=f"ssq_{jc}", bufs=4
                        )
                        st["ssq"] = ssq
                        nc.vector.tensor_mul(sq[:], xc[:], xc[:])
                        nc.vector.reduce_sum(ssq[:], sq[:], axis=mybir.AxisListType.X)

                    return [s1, s2]

                def ln_finish(jc):
                    """Sqrt + scale/shift + output DMA for chunk jc (tail)."""
                    xc = ln_st[jc]["xc"]
                    ssq = ln_st[jc]["ssq"]
                    sd = spool.tile([P, 1], F32, tag="sd", name=f"sd_{jc}", bufs=2)
                    rstd = spool.tile(
                        [P, 1], F32, tag="rstd", name=f"rstd_{jc}", bufs=2
                    )
                    y = spool.tile([P, D], F32, tag="y", name=f"y_{jc}", bufs=2)
                    nc.scalar.activation(
                        sd[:], ssq[:], Sqrt, scale=1.0 / D, bias=eps_sb[:]
                    )
                    nc.vector.reciprocal(rstd[:], sd[:])
                    nc.vector.tensor_scalar_mul(y[:], xc[:], rstd[:])
                    nc.vector.tensor_mul(y[:], y[:], gam_sb[:])
                    nc.vector.tensor_add(y[:], y[:], bet_sb[:])
                    nc.sync.dma_start(out.ap()[jc], y[:])

                def attn_chunk(jc, fillers):
                    # prefetch next chunk's x_q slice + this chunk's residual
                    if jc + 1 < NQC:
                        xq_rings[jc + 1] = xq_fetch(jc + 1)
                    qres_rings[jc] = qres_fetch(jc)
                    ct = spool.tile(
                        [P, B, GSZ, P], BF16, tag="ct", name=f"ct_{jc}", bufs=2
                    )
                    cc_in = dram.tile([NCORES * P, P], BF16, name=f"cc_in_{jc}")
                    cc_in_v = cc_in.rearrange("(d p) q -> p d q", p=P)
                    qt_r = qt_rings[jc]
                    for b in range(B):
                        ctx = psB.tile(
                            [P, QCH], F32, tag="ctx", name=f"ctx_{jc}_{b}", bufs=2
                        )
                        dn = psB.tile(
                            [P, QCH], F32, tag="dn", name=f"dn_{jc}_{b}", bufs=1
                        )

                        def scores(kt):
                            # one kt (both heads) per 2-bank psum slab; slabs
                            # ping-pong (tags sA/sB) so the next kt's matmuls
                            # run while ACT still exps the previous slab.
                            j = kt % 2
                            s2 = psB.tile(
                                [P, 2, QCH],
                                F32,
                                tag=("sA", "sB")[j],
                                name=f"s_{jc}_{b}_{kt}",
                                bufs=1,
                            )
                            ksl = slice(P * kt, P * kt + P)
                            nc.tensor.matmul(
                                s2[:, 0, :],
                                lhsT=kt_sb[0:DK, b, ksl],
                                rhs=qt_r[0:DK, b, :],
                            )
                            anchor["pe"] = nc.tensor.matmul(
                                s2[:, 1, :],
                                lhsT=kt_sb[DK:P, b, ksl],
                                rhs=qt_r[DK:P, b, :],
                            )
                            nc.scalar.activation(
                                p_sb[:, kt // 2, 2 * j : 2 * j + 2], s2[:], Exp
                            )

                        def ctx2(kt):
                            ktp, j = kt // 2, kt % 2
                            st_, sp = kt == 0, kt == NKT - 1
                            nc.tensor.matmul(
                                ctx[0:DK, :],
                                lhsT=v_sb[:, b, kt, 0:DK],
                                rhs=p_sb[:, ktp, 2 * j, :],
                                start=st_,
                                stop=sp,
                            )
                            nc.tensor.matmul(
                                ctx[DK:P, :],
                                lhsT=v_sb[:, b, kt, DK:P],
                                rhs=p_sb[:, ktp, 2 * j + 1, :],
                                start=st_,
                                stop=sp,
                            )
                            # denominator rows, broadcast across the head's 64
                            # partitions by the ones stationary (same
                            # 512-cycle stream as a 1-row output)
                            nc.tensor.matmul(
                                dn[0:DK, :],
                                lhsT=onesP[:, 0:DK],
                                rhs=p_sb[:, ktp, 2 * j, :],
                                start=st_,
                                stop=sp,
                            )
                            anchor["pe"] = nc.tensor.matmul(
                                dn[DK:P, :],
                                lhsT=onesP[:, DK:P],
                                rhs=p_sb[:, ktp, 2 * j + 1, :],
                                start=st_,
                                stop=sp,
                            )

                        # fillers pop between the scores pair (which feeds the
                        # ACT exp stream) and the ctx/dn accumulation, so they
                        # bridge the boundary reciprocal-WAR stall on dn
                        # without delaying the next exp.
                        scores(0)
                        scores(1)
                        if b == 1:
                            for _ in range(2):
                                if fillers:
                                    fillers.pop(0)()
                        for kt2 in range(2, NKT, 2):
                            scores(kt2)
                            scores(kt2 + 1)
                            if (kt2 >= 4 or kt2 == 2) and fillers:
                                fillers.pop(0)()
                            ctx2(kt2 - 2)
                            ctx2(kt2 - 1)
                        ctx2(NKT - 2)
                        ctx2(NKT - 1)

                        # normalize: ct = ctx * (1/dn), then ship to DRAM for
                        # the A2A (dn rows are already broadcast per head)
                        rd = spool.tile(
                            [P, QCH], F32, tag="rd", name=f"rd_{jc}_{b}", bufs=2
                        )
                        nc.vector.reciprocal(rd[:], dn[:])
                        nc.vector.tensor_mul(ct[:, b], ctx[:], rd[:])
                        nc.sync.dma_start(
                            cc_in_v[:, GSZ * b : GSZ * b + GSZ, :], ct[:, b]
                        )

                    cc_out[jc] = dram.tile([NCORES * P, P], BF16, name=f"cc_out_{jc}")
                    nc.gpsimd.collective_compute(
                        "AllToAll",
                        mybir.AluOpType.bypass,
                        replica_groups=groups,
                        ins=[cc_in.opt()],
                        outs=[cc_out[jc].opt()],
                    )

                    while fillers:
                        fillers.pop(0)()

                # Q'(0) before the first chunk
                for step in make_qproj_steps(0):
                    step()

                def make_warm_steps(jc, n):
                    # spare PE work for thin early chunks: keeps HAM at full
                    # clock across the softmax-reciprocal boundary stalls
                    def wstep(i):
                        wm = psB.tile(
                            [P, QCH], F32, tag="fill", name=f"wf_{jc}_{i}", bufs=1
                        )
                        for _ in range(2):
                            pin(
                                nc.tensor.matmul(
                                    wm[:], lhsT=onesP[:], rhs=wo_sb[:, 0, 0:QCH]
                                )
                            )

                    return [(lambda i=i: wstep(i)) for i in range(n)]

                for jc in range(NQC):
                    # o-proj for chunk jc-2: its A2A has had a full chunk to
                    # complete, so these steps never stall the PE queue even
                    # when a collective runs long.
                    fillers = []
                    if jc >= 2:
                        fillers += make_oproj_steps(jc - 2)
                    else:
                        # early pops must not touch the just-issued xq DMA
                        fillers += make_warm_steps(jc, 2)
                    if jc < NQC - 1:
                        fillers += make_qproj_steps(jc + 1)
                    if jc >= 2:
                        fillers += make_ln_stat_steps(jc - 2)
                    else:
                        fillers += make_warm_steps(jc + 10, 3)
                    attn_chunk(jc, fillers)

                # ---- tail ----
                # chunk 2's o-proj (A2A long done) + LN finishes overlap the
                # last chunk's A2A; keep-warm matmuls hold the PE at full
                # clock across the collective wait so o-proj(3) runs warm.
                ln_finish(0)
                ln_finish(1)
                for step in make_oproj_steps(2):
                    step()
                for step in make_ln_stat_steps(2):
                    step()
                ln_finish(2)
                for i in range(28):
                    wm = psB.tile(
                        [P, QCH], F32, tag="fill", name=f"warmmm_{i}", bufs=1
                    )
                    pin(
                        nc.tensor.matmul(
                            wm[:], lhsT=onesP[:], rhs=wo_sb[:, 0, 0:QCH]
                        )
                    )
                for step in make_oproj_steps(NQC - 1):
                    step()
                for step in make_ln_stat_steps(NQC - 1):
                    step()
                ln_finish(NQC - 1)

    _split_waits(nc)
    return nc


def _prep_inputs(query, key_value, W_qkv, b_qkv, W_o, b_o, ln_gamma, ln_beta):
    bf16 = ml_dtypes.bfloat16
    f32 = np.float32
    query = np.asarray(query, f32)
    key_value = np.asarray(key_value, f32)
    W_qkv = np.asarray(W_qkv, f32)
    b_qkv = np.asarray(b_qkv, f32)
    W_o = np.asarray(W_o, f32)
    b_o = np.asarray(b_o, f32)
    ln_gamma = np.asarray(ln_gamma, f32)
    ln_beta = np.asarray(ln_beta, f32)

    Wq, Wk, Wv = W_qkv[:D], W_qkv[D : 2 * D], W_qkv[2 * D :]
    bq, bk, bv = b_qkv[:D], b_qkv[D : 2 * D], b_qkv[2 * D :]

    woT_full = np.ascontiguousarray(W_o.T).astype(bf16)  # [d_in, n_out]
    gam = np.ascontiguousarray(np.broadcast_to(ln_gamma, (P, D))).astype(f32)
    bet = np.ascontiguousarray(np.broadcast_to(ln_beta, (P, D))).astype(f32)

    xqT = np.ascontiguousarray(query.transpose(0, 2, 1)).astype(bf16)
    xkvT = np.ascontiguousarray(key_value.transpose(0, 2, 1)).astype(bf16)

    in_maps = []
    for c in range(NCORES):
        b = c // GSZ
        jb = c % GSZ
        sl = slice(DLOC * c, DLOC * c + DLOC)  # this core's 2 heads
        # this core owns q rows 512*jc + 128*jb .. +128 of batch b
        res_rows = np.stack(
            [
                query[b, QCH * jc + P * jb : QCH * jc + P * jb + P] + b_o[None, :]
                for jc in range(NQC)
            ]
        )
        in_maps.append(
            {
                "xqT": xqT,
                "xkvT": xkvT,
                "wqT": np.ascontiguousarray(Wq[sl].T).astype(bf16),
                "wkT": np.ascontiguousarray(Wk[sl].T).astype(bf16),
                "wvT": np.ascontiguousarray(Wv[sl].T).astype(bf16),
                "bqs": (bq[sl] * 0.125)[:, None].astype(f32),
                "bks": bk[sl][:, None].astype(f32),
                "bvr": bv[sl][None, :].astype(bf16),
                "woT": woT_full,
                "qres": res_rows.astype(f32),
                "gam": gam,
                "bet": bet,
            }
        )
    return in_maps


def kernel(query, key_value, W_qkv, b_qkv, W_o, b_o, ln_gamma, ln_beta):
    global LAST_RESULT
    if "nc" not in _CACHE:
        _CACHE["nc"] = _build()
    nc = _CACHE["nc"]
    in_maps = _prep_inputs(
        query, key_value, W_qkv, b_qkv, W_o, b_o, ln_gamma, ln_beta
    )
    res = run_bass_kernel_spmd(nc, in_maps, core_ids=list(range(NCORES)))
    LAST_RESULT = res
    full = np.empty((B, SQ, D), np.float32)
    for c in range(NCORES):
        b = c // GSZ
        jb = c % GSZ
        o = res.results[c]["out"]  # [NQC, P, D]
        for jc in range(NQC):
            r0 = QCH * jc + P * jb
            full[b, r0 : r0 + P] = o[jc]
    return full


# revision 55
# speedup vs baseline: 1.0754x; 1.0151x over previous
"""Memory-efficient multi-head cross-attention on 8 TRN2 NeuronCores.

Sharding: each core owns 2 heads ({2c, 2c+1}) for BOTH batches
(tensor-parallel qkv projections over the head axis).  Per 512-row q chunk,
each core normalizes its context and a single dense 8-core AllToAll exchanges
[head-block x (batch, q-block)] tiles, after which every core holds the full
1024-channel context for its own (batch c//4, q rows 512*jc + 128*(c%4)).
The o-projection, residual add and LayerNorm then run fully locally -- no
cross-core reduction of o-proj partials is needed.

Pipeline (per chunk jc): attention(jc) emits Q'proj(jc+1), LN-stats(jc-2) and
o-proj(jc-1) as PE/DVE filler inside the (ACT-exp-bound) attention inner loop;
the A2A for chunk jc triggers as soon as both batches' context is normalized.
LayerNorm's Sqrt-dependent finish runs at the tail to avoid ACT table-set
switches between Exp and Sqrt.

kernel(**inputs) takes the FULL unsharded inputs and returns the FULL output.
"""

import sys
import types

import ml_dtypes
import numpy as np

# ---------------------------------------------------------------------------
# Environment shims (must run before concourse imports are used)
# ---------------------------------------------------------------------------


def _install_ntff_shim():
    """Provide antenv.axon_hooks (absent in this image) so that
    run_bass_kernel_spmd(trace=True) can capture NTFF profiles via the
    axon ctypes hook. Harmless when tracing is off."""
    if "antenv.axon_hooks" in sys.modules:
        return
    hook = None
    try:
        from trn_agent_boot.trn_boot import _ntff_profile_via_ctypes

        hook = _ntff_profile_via_ctypes("/opt/axon/libaxon_pjrt.so")
    except Exception:
        hook = None
    mod = types.ModuleType("antenv.axon_hooks")
    mod.get_axon_ntff_profile_hook = lambda: hook
    mod.set_axon_ntff_profile_hook = lambda h: None
    sys.modules["antenv.axon_hooks"] = mod


_install_ntff_shim()

import concourse.bass as bass  # noqa: E402
import concourse.mybir as mybir  # noqa: E402
import concourse.tile as tile  # noqa: E402
from concourse.bass_utils import run_bass_kernel_spmd  # noqa: E402
from concourse.vector_clock import ScopedClock  # noqa: E402
import bass_rust as _bass_rust  # noqa: E402


def _patched_drain_and_barrier(self, tick_clock, wait_clock):
    """The walrus build in this image rejects a Drain carrying multiple sem
    waits ("Too many sync wait commands").  Emit the kernel-tail waits as
    standalone wait instructions on the sync engine instead, then drain."""
    nc = self.nc
    probe = nc.sync.nop(nofuse=True)
    wait_clock.add_sem_waits(probe.ins, ScopedClock({None: tick_clock.global_clock}))
    waits = list(probe.ins.sync_info.on_wait)
    probe.ins.sync_info.on_wait.clear()
    name2sem = {s.name: s for s in self.sems.allocated().values()}
    for w in waits:
        nc.sync.wait_ge(name2sem[w.ant_name], w.wait_value)
    nc.sync.drain()
    nc.all_engine_barrier()
    popped = nc._tile_sem_poison_stack.pop()
    assert popped is self._sem_poison
    nc.clear_and_free_semaphores(list(self.sems.allocated().values()))
    nc.all_engine_barrier()


tile.TileContext._drain_and_barrier = _patched_drain_and_barrier

# Max sem-waits this walrus build accepts on a single instruction.
_WAIT_LIMIT = 1


def _split_waits(nc, limit=_WAIT_LIMIT):
    """Hoist excess per-instruction sem waits into standalone EventSemaphore
    instructions (same engine, immediately preceding), since this walrus build
    rejects instructions carrying more than one sync wait."""
    n_split = 0
    for f in nc.m.functions:
        for bb in f.blocks:
            insts = bb.instructions
            i = 0
            while i < len(insts):
                inst = insts[i]
                si = getattr(inst, "sync_info", None)
                waits = si.on_wait if si is not None else None
                if waits is not None and len(waits) > limit:
                    excess = list(waits)[limit:]
                    del waits[limit:]
                    for w in excess:
                        ev = mybir.InstEventSemaphore(
                            name=f"I-{nc.next_id()}",
                            engine=inst.engine,
                            ins=[],
                            outs=[],
                        )
                        ev.sync_info = mybir.SyncInfo(on_wait=[w], on_update=[])
                        insts.insert(i, ev)
                        i += 1
                        n_split += 1
                i += 1
    return n_split


# ---------------------------------------------------------------------------
# Problem constants (hardcoded per the harness contract)
# ---------------------------------------------------------------------------
B = 2
SQ = 2048
SKV = 2048
D = 1024
NH = 16
DK = 64

NCORES = 8
GSZ = 4  # q-blocks per chunk / cores per batch in the output mapping
DLOC = 2 * DK  # 128 local context channels (2 heads)
P = 128
QCH = 512  # q chunk (matmul moving free dim)
NQC = SQ // QCH  # 4
NKT = SKV // P  # 16 k tiles
NMT = D // P  # 8 contraction tiles over model dim

F32 = mybir.dt.float32
BF16 = mybir.dt.bfloat16

LN_EPS = 1e-5

_CACHE = {}
LAST_RESULT = None


def _build():
    """Build the SPMD Bass program (identical on all 8 cores)."""
    nc = bass.Bass("TRN2", target_bir_lowering=False, num_devices=NCORES)

    # ---- kernel I/O (per-core shards supplied by the host) ----
    # x inputs are full (both batches), transposed: [b, d, seq]
    xqT = nc.dram_tensor("xqT", [B, D, SQ], BF16, kind="ExternalInput")
    xkvT = nc.dram_tensor("xkvT", [B, D, SKV], BF16, kind="ExternalInput")
    # projection weights for this core's 2 heads: [d_in, 128]
    wqT = nc.dram_tensor("wqT", [D, DLOC], BF16, kind="ExternalInput")
    wkT = nc.dram_tensor("wkT", [D, DLOC], BF16, kind="ExternalInput")
    wvT = nc.dram_tensor("wvT", [D, DLOC], BF16, kind="ExternalInput")
    bqs = nc.dram_tensor("bqs", [P, 1], F32, kind="ExternalInput")
    bks = nc.dram_tensor("bks", [P, 1], F32, kind="ExternalInput")
    bvr = nc.dram_tensor("bvr", [1, DLOC], BF16, kind="ExternalInput")
    # full o-proj weights W_o.T [d_in, n]; rows 128*s.. = heads {2s, 2s+1}
    woT = nc.dram_tensor("woT", [D, D], BF16, kind="ExternalInput")
    # residual rows (query + b_o) for this core: [jc, 128, D]
    qres = nc.dram_tensor("qres", [NQC, P, D], F32, kind="ExternalInput")
    gam = nc.dram_tensor("gam", [P, D], F32, kind="ExternalInput")
    bet = nc.dram_tensor("bet", [P, D], F32, kind="ExternalInput")
    out = nc.dram_tensor("out", [NQC, P, D], F32, kind="ExternalOutput")

    groups = [[0, 1, 2, 3, 4, 5, 6, 7]]
    Exp = mybir.ActivationFunctionType.Exp
    Sqrt = mybir.ActivationFunctionType.Sqrt
    MULT = mybir.AluOpType.mult
    ADD = mybir.AluOpType.add

    xq_v = xqT.ap().rearrange("b (t p) q -> p b t q", p=P)
    xkv_v = xkvT.ap().rearrange("b (t p) q -> p b t q", p=P)

    with tile.TileContext(nc) as tc:
        with (
            tc.tile_pool(name="cpool", bufs=1) as cpool,
            tc.tile_pool(name="spool", bufs=2) as spool,
            tc.tile_pool(name="dram", bufs=1, space="DRAM") as dram,
        ):
            # ---- persistent SBUF tensors ----
            wq_sb = cpool.tile([P, NMT, DLOC], BF16)
            wk_sb = cpool.tile([P, NMT, DLOC], BF16)
            wv_sb = cpool.tile([P, NMT, DLOC], BF16)
            bqs_sb = cpool.tile([P, 1], F32)
            bks_sb = cpool.tile([P, 1], F32)
            bvr_sb = cpool.tile([1, DLOC], BF16)
            onesP = cpool.tile([P, P], BF16)
            eps_sb = cpool.tile([P, 1], F32)
            # K'^T per batch (d-of-head-pair on partitions)
            kt_sb = cpool.tile([P, B, SKV], BF16)
            v_sb = cpool.tile([P, B, NKT, DLOC], BF16)  # V rows (k on partitions)
            wo_sb = cpool.tile([P, NMT, D], BF16)  # W_o^T, slab s = heads {2s,2s+1}
            gam_sb = cpool.tile([P, D], F32)
            bet_sb = cpool.tile([P, D], F32)

            # Queue budget: sync+scalar carry the 8MB xkv stream (split); the
            # gpsimd queue issues the secondary loads as doorbells before the
            # warmup collective's completion wait blocks it.
            nc.sync.dma_start(wk_sb[:], wkT.ap().rearrange("(t p) d -> p t d", p=P))
            nc.sync.dma_start(bks_sb[:], bks.ap())
            nc.scalar.dma_start(wv_sb[:], wvT.ap().rearrange("(t p) d -> p t d", p=P))
            nc.scalar.dma_start(bvr_sb[:], bvr.ap())
            nc.vector.memset(onesP[:], 1.0)
            nc.vector.memset(eps_sb[:], LN_EPS)

            # xq chunk ring: [P, b, mt, q] for one chunk
            def xq_fetch(jc):
                xq_r = spool.tile(
                    [P, B, NMT, QCH], BF16, tag="xqr", name=f"xqr_{jc}", bufs=1
                )
                nc.sync.dma_start(
                    xq_r[:], xq_v[:, :, :, QCH * jc : QCH * jc + QCH]
                )
                return xq_r

            # residual rows ring: [P, D] for one chunk
            def qres_fetch(jc):
                qres_r = spool.tile(
                    [P, D], F32, tag="qres", name=f"qres_{jc}", bufs=3
                )
                nc.sync.dma_start(qres_r[:], qres.ap()[jc])
                return qres_r

            # ---------------- Phase A: K'/V projections ----------------
            # Warm up the collectives stream early: a tiny AllToAll during
            # phase A absorbs the ~30us first-collective ncfw startup cost
            # that would otherwise stall the chunk-1 o-proj pipeline.
            warm_sb = cpool.tile([P, NCORES, P], BF16)
            nc.vector.memset(warm_sb[:], 0.0)
            warm_in = dram.tile([NCORES * P, P], BF16, name="warm_in")
            warm_out = dram.tile([NCORES * P, P], BF16, name="warm_out")
            nc.gpsimd.dma_start(
                warm_in.rearrange("(s p) q -> p s q", p=P), warm_sb[:]
            )
            nc.gpsimd.collective_compute(
                "AllToAll",
                mybir.AluOpType.bypass,
                replica_groups=[[0, 1, 2, 3, 4, 5, 6, 7]],
                ins=[warm_in.opt()],
                outs=[warm_out.opt()],
            )

            xq_rings = {}
            with (
                tc.tile_pool(name="apool", bufs=1) as apool,
                tc.tile_pool(name="psA", bufs=8, space="PSUM") as psA,
            ):
                # spread the bulk xkv stream over two engines' hardware DMA
                # queues -- a single engine's queue tops out well below HBM
                # bandwidth
                xkv_tiles = []
                last_dma = {}
                for b in range(B):
                    xkv_sb = apool.tile(
                        [P, NMT, SKV], BF16, tag="xkv", name=f"xkv_{b}", bufs=2
                    )
                    xkv_tiles.append(xkv_sb)
                    for mt in range(NMT):
                        eng = "sync" if mt % 2 == 0 else "scalar"
                        last_dma[eng] = getattr(nc, eng).dma_start(
                            xkv_sb[:, mt, :], xkv_v[:, b, mt, :]
                        )
                # secondary loads behind the xkv stream, order pinned so the
                # scheduler cannot hoist their transfers ahead of it
                xq_rings[0] = xq_fetch(0)
                for dma in (
                    nc.scalar.dma_start(
                        wq_sb[:], wqT.ap().rearrange("(t p) d -> p t d", p=P)
                    ),
                    nc.scalar.dma_start(bqs_sb[:], bqs.ap()),
                    nc.scalar.dma_start(
                        wo_sb[:], woT.ap().rearrange("(t p) n -> p t n", p=P)
                    ),
                    nc.scalar.dma_start(gam_sb[:], gam.ap()),
                    nc.scalar.dma_start(bet_sb[:], bet.ap()),
                ):
                    _bass_rust.add_dep_helper(
                        dma.ins, last_dma["scalar"].ins, False, "dma order"
                    )
                for b in range(B):
                    xkv_sb = xkv_tiles[b]

                    # K'^T: out[dloc(128), kv(512)], m-tile outer so matmuls
                    # start as soon as the first input chunk lands.
                    pss = [
                        psA.tile([P, QCH], F32, tag="pj", name=f"pk_{b}_{i}")
                        for i in range(NQC)
                    ]
                    for mt in range(NMT):
                        for qc in range(NQC):
                            nc.tensor.matmul(
                                pss[qc][:],
                                lhsT=wk_sb[:, mt, :],
                                rhs=xkv_sb[:, mt, QCH * qc : QCH * qc + QCH],
                                start=(mt == 0),
                                stop=(mt == NMT - 1),
                            )
                    for qc in range(NQC):
                        nc.vector.tensor_scalar(
                            kt_sb[:, b, QCH * qc : QCH * qc + QCH],
                            pss[qc][:],
                            1.0,
                            bks_sb[:, 0:1],
                            MULT,
                            ADD,
                        )

                    # V: out[k_tile(128), dloc(128)] over m-tiles + bias row.
                    for kt in range(NKT):
                        ps = psA.tile([P, QCH], F32, tag="pj", name=f"pv_{b}_{kt}")
                        pv = ps[:, :DLOC]
                        for mt in range(NMT):
                            nc.tensor.matmul(
                                pv,
                                lhsT=xkv_sb[:, mt, P * kt : P * kt + P],
                                rhs=wv_sb[:, mt, :],
                                start=(mt == 0),
                                stop=False,
                            )
                        nc.tensor.matmul(
                            pv,
                            lhsT=onesP[0:1, :],
                            rhs=bvr_sb[0:1, :],
                            start=False,
                            stop=True,
                        )
                        nc.vector.tensor_copy(v_sb[:, b, kt, :], pv)

            # ------- Phase B: per-chunk attention + A2A + o-proj + LN -------
            with (
                tc.tile_pool(name="bpool", bufs=1) as bpool,
                tc.tile_pool(name="psB", bufs=1, space="PSUM") as psB,
            ):
                # exp(scores): [k(128), ktp, head*kt-in-pair, q]
                p_sb = bpool.tile([P, NKT // 2, 4, QCH], BF16)

                # The tile scheduler reorders instructions; filler steps must
                # be pinned behind the attention matmuls they are meant to
                # pad, or they get hoisted to the chunk start where their
                # input waits (collectives, DMAs) block the in-order queues.
                anchor = {}

                def pin(inst):
                    if "pe" in anchor:
                        _bass_rust.add_dep_helper(
                            inst.ins, anchor["pe"].ins, False, "filler order"
                        )
                    return inst

                cc_out = {}
                qt_rings = {}
                qres_rings = {}
                x_tiles = {}
                ln_st = {}

                def make_qproj_steps(jc):
                    """Q'^T projection for chunk jc: 4 filler steps."""
                    st = {}
                    qt_rings[jc] = spool.tile(
                        [P, B, QCH], BF16, tag="qt", name=f"qt_{jc}", bufs=2
                    )

                    def mm_step(b, half):
                        if half == 0:
                            st[b] = psB.tile(
                                [P, QCH], F32, tag="fill", name=f"pq_{jc}_{b}", bufs=1
                            )
                        pq = st[b]
                        xq_r = xq_rings[jc]
                        for mt in range(4 * half, 4 * half + 4):
                            pin(
                                nc.tensor.matmul(
                                    pq[:],
                                    lhsT=wq_sb[:, mt, :],
                                    rhs=xq_r[:, b, mt, :],
                                    start=(mt == 0),
                                    stop=(mt == NMT - 1),
                                )
                            )
                        if half == 1:
                            nc.vector.tensor_scalar(
                                qt_rings[jc][:, b, :],
                                pq[:],
                                0.125,
                                bqs_sb[:, 0:1],
                                MULT,
                                ADD,
                            )

                    return [
                        (lambda b=b, h=h: mm_step(b, h))
                        for b in range(B)
                        for h in range(2)
                    ]

                def make_oproj_steps(jc):
                    """A2A receive + o-projection for chunk jc: 5 steps."""
                    st = {}

                    def dmain_step():
                        # Mid-stream (jc<=1): scalar queue -- its HWDGE
                        # doorbell is ~10x cheaper than gpsimd's SWDGE build,
                        # and the cc_out it waits on is a 2-chunk-old
                        # collective, so it never stalls the exps behind it.
                        # Tail (jc>=2): gpsimd queue -- the last A2A is still
                        # in flight there, and a scalar-queue doorbell waiting
                        # on it would block the LayerNorm sqrts behind it.
                        ctg = spool.tile(
                            [P, NCORES, P], BF16, tag="ctg", name=f"ctg_{jc}", bufs=2
                        )
                        st["ctg"] = ctg
                        if jc >= 2:
                            # sync queue: HWDGE doorbell (~0.6us vs gpsimd's
                            # ~6us SWDGE build); only the final out-DMAs sit
                            # behind its collective wait, and they are not
                            # latency-critical.
                            nc.sync.dma_start(
                                ctg[:], cc_out[jc].rearrange("(s p) q -> p s q", p=P)
                            )
                        else:
                            pin(
                                nc.scalar.dma_start(
                                    ctg[:],
                                    cc_out[jc].rearrange("(s p) q -> p s q", p=P),
                                )
                            )

                    def mm_step(nch, half):
                        if nch == 0 and half == 0:
                            x_tiles[jc] = spool.tile(
                                [P, D], F32, tag="x", name=f"x_{jc}", bufs=2
                            )
                        if half == 0:
                            st["po"] = psB.tile(
                                [P, QCH], F32, tag="fill", name=f"po_{jc}_{nch}", bufs=1
                            )
                        po = st["po"]
                        ctg = st["ctg"]
                        nsl = slice(QCH * nch, QCH * nch + QCH)
                        for s in range(4 * half, 4 * half + 4):
                            pin(
                                nc.tensor.matmul(
                                    po[:],
                                    lhsT=ctg[:, s],
                                    rhs=wo_sb[:, s, nsl],
                                    start=(s == 0),
                                    stop=(s == NCORES - 1),
                                )
                            )
                        if half == 1:
                            nc.vector.tensor_add(
                                x_tiles[jc][:, nsl], po[:], qres_rings[jc][:, nsl]
                            )

                    return [dmain_step] + [
                        (lambda n=n, h=h: mm_step(n, h))
                        for n in range(2)
                        for h in range(2)
                    ]

                def make_ln_stat_steps(jc):
                    """LayerNorm statistics for chunk jc (DVE only): 2 steps."""
                    st = {}
                    ln_st[jc] = st

                    def s1():
                        x = x_tiles[jc]
                        mean = spool.tile(
                            [P, 1], F32, tag="mean", name=f"mean_{jc}", bufs=4
                        )
                        nmean = spool.tile(
                            [P, 1], F32, tag="nmean", name=f"nmean_{jc}", bufs=4
                        )
                        xc = spool.tile([P, D], F32, tag="xc", name=f"xc_{jc}", bufs=3)
                        st["xc"] = xc
                        pin(
                            nc.vector.reduce_sum(
                                mean[:], x[:], axis=mybir.AxisListType.X
                            )
                        )
                        nc.vector.tensor_scalar_mul(nmean[:], mean[:], -1.0 / D)
                        nc.vector.tensor_scalar_add(xc[:], x[:], nmean[:])

                    def s2():
                        xc = st["xc"]
                        sq = spool.tile([P, D], F32, tag="sq", name=f"sq_{jc}", bufs=1)
                        ssq = spool.tile(
                            [P, 1], F32, tag="ssq", name=f"ssq_{jc}", bufs=4
                        )
                        st["ssq"] = ssq
                        nc.vector.tensor_mul(sq[:], xc[:], xc[:])
                        nc.vector.reduce_sum(ssq[:], sq[:], axis=mybir.AxisListType.X)

                    return [s1, s2]

                def ln_finish(jc):
                    """Sqrt + scale/shift + output DMA for chunk jc (tail)."""
                    xc = ln_st[jc]["xc"]
                    ssq = ln_st[jc]["ssq"]
                    sd = spool.tile([P, 1], F32, tag="sd", name=f"sd_{jc}", bufs=2)
                    rstd = spool.tile(
                        [P, 1], F32, tag="rstd", name=f"rstd_{jc}", bufs=2
                    )
                    y = spool.tile([P, D], F32, tag="y", name=f"y_{jc}", bufs=2)
                    nc.scalar.activation(
                        sd[:], ssq[:], Sqrt, scale=1.0 / D, bias=eps_sb[:]
                    )
                    nc.vector.reciprocal(rstd[:], sd[:])
                    nc.vector.tensor_scalar_mul(y[:], xc[:], rstd[:])
                    nc.vector.tensor_mul(y[:], y[:], gam_sb[:])
                    nc.vector.tensor_add(y[:], y[:], bet_sb[:])
                    nc.sync.dma_start(out.ap()[jc], y[:])

                def attn_chunk(jc, fillers):
                    # prefetch next chunk's x_q slice + this chunk's residual
                    if jc + 1 < NQC:
                        xq_rings[jc + 1] = xq_fetch(jc + 1)
                    qres_rings[jc] = qres_fetch(jc)
                    ct = spool.tile(
                        [P, B, GSZ, P], BF16, tag="ct", name=f"ct_{jc}", bufs=2
                    )
                    cc_in = dram.tile([NCORES * P, P], BF16, name=f"cc_in_{jc}")
                    cc_in_v = cc_in.rearrange("(d p) q -> p d q", p=P)
                    qt_r = qt_rings[jc]
                    for b in range(B):
                        ctx = psB.tile(
                            [P, QCH], F32, tag="ctx", name=f"ctx_{jc}_{b}", bufs=2
                        )
                        dn = psB.tile(
                            [P, QCH], F32, tag="dn", name=f"dn_{jc}_{b}", bufs=1
                        )

                        def scores(kt):
                            # one kt (both heads) per 2-bank psum slab; slabs
                            # ping-pong (tags sA/sB) so the next kt's matmuls
                            # run while ACT still exps the previous slab.
                            j = kt % 2
                            s2 = psB.tile(
                                [P, 2, QCH],
                                F32,
                                tag=("sA", "sB")[j],
                                name=f"s_{jc}_{b}_{kt}",
                                bufs=1,
                            )
                            ksl = slice(P * kt, P * kt + P)
                            nc.tensor.matmul(
                                s2[:, 0, :],
                                lhsT=kt_sb[0:DK, b, ksl],
                                rhs=qt_r[0:DK, b, :],
                            )
                            anchor["pe"] = nc.tensor.matmul(
                                s2[:, 1, :],
                                lhsT=kt_sb[DK:P, b, ksl],
                                rhs=qt_r[DK:P, b, :],
                            )
                            nc.scalar.activation(
                                p_sb[:, kt // 2, 2 * j : 2 * j + 2], s2[:], Exp
                            )

                        def ctx2(kt):
                            ktp, j = kt // 2, kt % 2
                            st_, sp = kt == 0, kt == NKT - 1
                            nc.tensor.matmul(
                                ctx[0:DK, :],
                                lhsT=v_sb[:, b, kt, 0:DK],
                                rhs=p_sb[:, ktp, 2 * j, :],
                                start=st_,
                                stop=sp,
                            )
                            nc.tensor.matmul(
                                ctx[DK:P, :],
                                lhsT=v_sb[:, b, kt, DK:P],
                                rhs=p_sb[:, ktp, 2 * j + 1, :],
                                start=st_,
                                stop=sp,
                            )
                            # denominator rows, broadcast across the head's 64
                            # partitions by the ones stationary (same
                            # 512-cycle stream as a 1-row output)
                            nc.tensor.matmul(
                                dn[0:DK, :],
                                lhsT=onesP[:, 0:DK],
                                rhs=p_sb[:, ktp, 2 * j, :],
                                start=st_,
                                stop=sp,
                            )
                            anchor["pe"] = nc.tensor.matmul(
                                dn[DK:P, :],
                                lhsT=onesP[:, DK:P],
                                rhs=p_sb[:, ktp, 2 * j + 1, :],
                                start=st_,
                                stop=sp,
                            )

                        # fillers pop between the scores pair (which feeds the
                        # ACT exp stream) and the ctx/dn accumulation, so they
                        # bridge the boundary reciprocal-WAR stall on dn
                        # without delaying the next exp.
                        scores(0)
                        scores(1)
                        if b == 1:
                            for _ in range(2):
                                if fillers:
                                    fillers.pop(0)()
                        for kt2 in range(2, NKT, 2):
                            scores(kt2)
                            scores(kt2 + 1)
                            if (kt2 >= 4 or kt2 == 2) and fillers:
                                fillers.pop(0)()
                            ctx2(kt2 - 2)
                            ctx2(kt2 - 1)
                        ctx2(NKT - 2)
                        ctx2(NKT - 1)

                        # normalize: ct = ctx * (1/dn), then ship to DRAM for
                        # the A2A (dn rows are already broadcast per head)
                        rd = spool.tile(
                            [P, QCH], F32, tag="rd", name=f"rd_{jc}_{b}", bufs=2
                        )
                        nc.vector.reciprocal(rd[:], dn[:])
                        nc.vector.tensor_mul(ct[:, b], ctx[:], rd[:])
                        nc.sync.dma_start(
                            cc_in_v[:, GSZ * b : GSZ * b + GSZ, :], ct[:, b]
                        )

                    cc_out[jc] = dram.tile([NCORES * P, P], BF16, name=f"cc_out_{jc}")
                    nc.gpsimd.collective_compute(
                        "AllToAll",
                        mybir.AluOpType.bypass,
                        replica_groups=groups,
                        ins=[cc_in.opt()],
                        outs=[cc_out[jc].opt()],
                    )

                    while fillers:
                        fillers.pop(0)()

                # Q'(0) before the first chunk
                for step in make_qproj_steps(0):
                    step()

                def make_warm_steps(jc, n):
                    # spare PE work for thin early chunks: keeps HAM at full
                    # clock across the softmax-reciprocal boundary stalls
                    def wstep(i):
                        wm = psB.tile(
                            [P, QCH], F32, tag="fill", name=f"wf_{jc}_{i}", bufs=1
                        )
                        for _ in range(2):
                            pin(
                                nc.tensor.matmul(
                                    wm[:], lhsT=onesP[:], rhs=wo_sb[:, 0, 0:QCH]
                                )
                            )

                    return [(lambda i=i: wstep(i)) for i in range(n)]

                for jc in range(NQC):
                    # o-proj for chunk jc-2: its A2A has had a full chunk to
                    # complete, so these steps never stall the PE queue even
                    # when a collective runs long.
                    fillers = []
                    if jc >= 2:
                        fillers += make_oproj_steps(jc - 2)
                    else:
                        # early pops must not touch the just-issued xq DMA
                        fillers += make_warm_steps(jc, 2)
                    if jc < NQC - 1:
                        fillers += make_qproj_steps(jc + 1)
                    if jc >= 2:
                        fillers += make_ln_stat_steps(jc - 2)
                    else:
                        fillers += make_warm_steps(jc + 10, 3)
                    attn_chunk(jc, fillers)

                # ---- tail ----
                # chunk 2's o-proj (A2A long done) + LN finishes overlap the
                # last chunk's A2A; keep-warm matmuls hold the PE at full
                # clock across the collective wait so o-proj(3) runs warm.
                ln_finish(0)
                ln_finish(1)
                for step in make_oproj_steps(2):
                    step()
                for step in make_ln_stat_steps(2):
                    step()
                ln_finish(2)
                for i in range(28):
                    wm = psB.tile(
                        [P, QCH], F32, tag="fill", name=f"warmmm_{i}", bufs=1
                    )
                    pin(
                        nc.tensor.matmul(
                            wm[:], lhsT=onesP[:], rhs=wo_sb[:, 0, 0:QCH]
                        )
                    )
                for step in make_oproj_steps(NQC - 1):
                    step()
                for step in make_ln_stat_steps(NQC - 1):
                    step()
                ln_finish(NQC - 1)

    _split_waits(nc)
    return nc


def _prep_inputs(query, key_value, W_qkv, b_qkv, W_o, b_o, ln_gamma, ln_beta):
    bf16 = ml_dtypes.bfloat16
    f32 = np.float32
    query = np.asarray(query, f32)
    key_value = np.asarray(key_value, f32)
    W_qkv = np.asarray(W_qkv, f32)
    b_qkv = np.asarray(b_qkv, f32)
    W_o = np.asarray(W_o, f32)
    b_o = np.asarray(b_o, f32)
    ln_gamma = np.asarray(ln_gamma, f32)
    ln_beta = np.asarray(ln_beta, f32)

    Wq, Wk, Wv = W_qkv[:D], W_qkv[D : 2 * D], W_qkv[2 * D :]
    bq, bk, bv = b_qkv[:D], b_qkv[D : 2 * D], b_qkv[2 * D :]

    woT_full = np.ascontiguousarray(W_o.T).astype(bf16)  # [d_in, n_out]
    gam = np.ascontiguousarray(np.broadcast_to(ln_gamma, (P, D))).astype(f32)
    bet = np.ascontiguousarray(np.broadcast_to(ln_beta, (P, D))).astype(f32)

    xqT = np.ascontiguousarray(query.transpose(0, 2, 1)).astype(bf16)
    xkvT = np.ascontiguousarray(key_value.transpose(0, 2, 1)).astype(bf16)

    in_maps = []
    for c in range(NCORES):
        b = c // GSZ
        jb = c % GSZ
        sl = slice(DLOC * c, DLOC * c + DLOC)  # this core's 2 heads
        # this core owns q rows 512*jc + 128*jb .. +128 of batch b
        res_rows = np.stack(
            [
                query[b, QCH * jc + P * jb : QCH * jc + P * jb + P] + b_o[None, :]
                for jc in range(NQC)
            ]
        )
        in_maps.append(
            {
                "xqT": xqT,
                "xkvT": xkvT,
                "wqT": np.ascontiguousarray(Wq[sl].T).astype(bf16),
                "wkT": np.ascontiguousarray(Wk[sl].T).astype(bf16),
                "wvT": np.ascontiguousarray(Wv[sl].T).astype(bf16),
                "bqs": (bq[sl] * 0.125)[:, None].astype(f32),
                "bks": bk[sl][:, None].astype(f32),
                "bvr": bv[sl][None, :].astype(bf16),
                "woT": woT_full,
                "qres": res_rows.astype(f32),
                "gam": gam,
                "bet": bet,
            }
        )
    return in_maps


def kernel(query, key_value, W_qkv, b_qkv, W_o, b_o, ln_gamma, ln_beta):
    global LAST_RESULT
    if "nc" not in _CACHE:
        _CACHE["nc"] = _build()
    nc = _CACHE["nc"]
    in_maps = _prep_inputs(
        query, key_value, W_qkv, b_qkv, W_o, b_o, ln_gamma, ln_beta
    )
    res = run_bass_kernel_spmd(nc, in_maps, core_ids=list(range(NCORES)))
    LAST_RESULT = res
    full = np.empty((B, SQ, D), np.float32)
    for c in range(NCORES):
        b = c // GSZ
        jb = c % GSZ
        o = res.results[c]["out"]  # [NQC, P, D]
        for jc in range(NQC):
            r0 = QCH * jc + P * jb
            full[b, r0 : r0 + P] = o[jc]
    return full
